# revision 1
# baseline (speedup 1.0000x reference)
"""Gammatone filterbank (4 cascaded complex one-pole IIR sections) on TRN2.

Algorithm (per waveform; all 128 bands in parallel on the 128 partitions):
  The complex recurrence s[t] = c*s[t-1] + u[t] with c = lam*e^{i*beta} is
  derotated per chunk: with sh[j] = s[t0+j]*e^{-i*j*beta} the recurrence
  becomes sh[j] = lam*sh[j-1] + u[t0+j]*e^{-i*j*beta} with REAL lam, so
  re/im decouple into independent real first-order scans on the DVE
  (tensor_tensor_scan).  The 4-stage cascade = 8 real scans per chunk.
  Mod/demod tables are chunk-local ([C, K], static in SBUF); each carries
  sqrt(factor) so the pair applies the stage-1 gain exactly once:
    Ar =  sqrt(f)*cos(j*b)*x,   Ai = -sqrt(f)*sin(j*b)*x
    (4x) Ar <- scan(lam, Ar),   Ai <- scan(lam, Ai)
    out = sqrt(f)*cos(j*b)*Yr + (-sqrt(f)*sin(j*b))*Yi = f*Re[cascade4(c,x)]
  Scan state carries across chunks via the scan `initial` operand; the
  carried complex state is rotated by e^{+i*K*beta} (per-channel constants
  cK/sK) to bridge chunk-local phase frames.

Engine split per chunk: DVE runs the 8 scans + the tiny batched state
rotation; GPSIMD runs the x partition-broadcast, the 2 modulation / 2
demodulation multiplies and the final add; the Scalar engine copies each
scan's last column into the state tile.  The broadcast+modulation for
chunk p+1 is emitted before chunk p's demodulation so GPSIMD feeds the
DVE ahead of time (software pipeline).  DMA traffic per core is just the
16 MB output + 128 KB input + one-time constants.

Sharding: batch-parallel SPMD, one waveform per NeuronCore (8 cores, B=8).
Output is [C, T] per core; the host transposes/stacks to [B, T, C].
"""

import sys

import numpy as np

for _p in ("/opt/trn_rl_repo",):
    if _p not in sys.path:
        sys.path.insert(0, _p)

import concourse.bass as bass  # noqa: F401
import concourse.mybir as mybir
from concourse.bacc import Bacc
from concourse.bass_utils import run_bass_kernel_spmd
from concourse.tile import TileContext

B = 8
T = 32000
C = 128
K = 2000          # time-chunk length (columns per DVE op)
NCHUNK = T // K
F32 = mybir.dt.float32
MULT = mybir.AluOpType.mult
ADD = mybir.AluOpType.add
SUB = mybir.AluOpType.subtract


def build_bass(t_len=T, k=K):
    nchunk = t_len // k
    assert nchunk * k == t_len
    nc = Bacc()
    x = nc.declare_dram_parameter("x", [1, t_len], F32, isOutput=False)
    mcl = nc.declare_dram_parameter("mcl", [C, k], F32, isOutput=False)
    msl = nc.declare_dram_parameter("msl", [C, k], F32, isOutput=False)
    lamt = nc.declare_dram_parameter("lamt", [C, k], F32, isOutput=False)
    ckp = nc.declare_dram_parameter("ck", [C, 1], F32, isOutput=False)
    skp = nc.declare_dram_parameter("sk", [C, 1], F32, isOutput=False)
    out = nc.declare_dram_parameter("out", [C, t_len], F32, isOutput=True)

    with TileContext(nc) as tc:
        with (
            tc.tile_pool(name="consts", bufs=1) as consts,
            tc.tile_pool(name="dmat", bufs=2) as dmat,
            tc.tile_pool(name="work", bufs=1) as work,
            tc.tile_pool(name="states", bufs=2) as stp,
        ):
            tabc = consts.tile([C, k], F32, tag="mcl", name="tabc")
            tabs = consts.tile([C, k], F32, tag="msl", name="tabs")
            lam_t = consts.tile([C, k], F32, tag="lam", name="lam_t")
            ck = consts.tile([C, 1], F32, tag="ck", name="ck")
            sk = consts.tile([C, 1], F32, tag="sk", name="sk")
            nc.sync.dma_start(out=tabc[:], in_=mcl[:])
            nc.sync.dma_start(out=tabs[:], in_=msl[:])
            nc.sync.dma_start(out=lam_t[:], in_=lamt[:])
            nc.sync.dma_start(out=ck[:], in_=ckp[:])
            nc.sync.dma_start(out=sk[:], in_=skp[:])

            def bcast_mod(p):
                """DMA x chunk p, broadcast it, modulate (GPSIMD)."""
                t0 = p * k
                xrow = dmat.tile([1, k], F32, tag="xrow", name="xrow")
                nc.sync.dma_start(out=xrow[:], in_=x[0:1, t0:t0 + k])
                xb = dmat.tile([C, k], F32, tag="xb", name="xb")
                nc.gpsimd.partition_broadcast(xb[:], xrow[:])
                mr = dmat.tile([C, k], F32, tag="Mr", name="mr")
                mi = dmat.tile([C, k], F32, tag="Mi", name="mi")
                nc.gpsimd.tensor_tensor(mr[:], tabc[:], xb[:], MULT)
                nc.gpsimd.tensor_tensor(mi[:], tabs[:], xb[:], MULT)
                return mr, mi

            # rotated initial states for the current chunk:
            # cols 0:4 = re(stage1..4), cols 4:8 = im(stage1..4)
            st_rot = stp.tile([C, 8], F32, tag="st_rot", name="st0")
            nc.vector.memset(st_rot[:], 0.0)

            mod_tiles = bcast_mod(0)
            for p in range(nchunk):
                t0 = p * k
                last = p == nchunk - 1
                mr, mi = mod_tiles

                # 4 cascaded one-pole stages = 8 real scans (DVE);
                # ACT saves each scan's last column as raw carried state.
                st_raw = stp.tile([C, 8], F32, tag="st_raw", name="st_raw")
                cur_r, cur_i = mr, mi
                for stage in range(4):
                    ab = stage % 2 == 0
                    nr = work.tile([C, k], F32, tag="Ar" if ab else "Br",
                                   name="nr", bufs=1 if ab else 2)
                    ni = work.tile([C, k], F32, tag="Ai" if ab else "Bi",
                                   name="ni", bufs=1 if ab else 2)
                    nc.vector.tensor_tensor_scan(
                        nr[:], lam_t[:], cur_r[:],
                        st_rot[:, stage:stage + 1], MULT, ADD)
                    nc.vector.tensor_tensor_scan(
                        ni[:], lam_t[:], cur_i[:],
                        st_rot[:, 4 + stage:5 + stage], MULT, ADD)
                    if not last:
                        nc.scalar.copy(
                            out=st_raw[:, stage:stage + 1],
                            in_=nr[:, k - 1:k])
                        nc.scalar.copy(
                            out=st_raw[:, 4 + stage:5 + stage],
                            in_=ni[:, k - 1:k])
                    cur_r, cur_i = nr, ni

                if not last:
                    # rotate carried state by e^{+i*K*beta} (batched, DVE):
                    # new_re = re*cK - im*sK ; new_im = im*cK + re*sK
                    tmp = stp.tile([C, 8], F32, tag="st_tmp", name="tmp")
                    nxt_rot = stp.tile([C, 8], F32, tag="st_rot", name="nxt")
                    nc.vector.tensor_scalar(
                        tmp[:, 0:4], st_raw[:, 4:8], sk[:], None, MULT)
                    nc.vector.tensor_scalar(
                        tmp[:, 4:8], st_raw[:, 0:4], sk[:], None, MULT)
                    nc.vector.scalar_tensor_tensor(
                        nxt_rot[:, 0:4], st_raw[:, 0:4], ck[:], tmp[:, 0:4],
                        MULT, SUB)
                    nc.vector.scalar_tensor_tensor(
                        nxt_rot[:, 4:8], st_raw[:, 4:8], ck[:], tmp[:, 4:8],
                        MULT, ADD)
                    st_rot = nxt_rot
                    # feed GPSIMD chunk p+1's inputs before demod of chunk p
                    mod_tiles = bcast_mod(p + 1)

                # demodulate (GPSIMD): z = tabc*Yr + tabs*Yi
                zr = work.tile([C, k], F32, tag="Dr", name="zr")
                zi = work.tile([C, k], F32, tag="Di", name="zi")
                z = dmat.tile([C, k], F32, tag="z", name="z")
                nc.gpsimd.tensor_tensor(zr[:], tabc[:], cur_r[:], MULT)
                nc.gpsimd.tensor_tensor(zi[:], tabs[:], cur_i[:], MULT)
                nc.gpsimd.tensor_tensor(z[:], zr[:], zi[:], ADD)

                nc.sync.dma_start(out=out[:, t0:t0 + k], in_=z[:])
    nc.finalize()
    return nc


def make_tables(coef_re, coef_im, factor, t_len=T, k=K):
    cr = np.asarray(coef_re, np.float64)
    ci = np.asarray(coef_im, np.float64)
    f = np.asarray(factor, np.float64)
    lam = np.hypot(cr, ci)
    beta = np.arctan2(ci, cr)
    sf = np.sqrt(f)
    j = np.arange(k, dtype=np.float64)
    ph = j[None, :] * beta[:, None]
    mcl = (sf[:, None] * np.cos(ph)).astype(np.float32)      # [C, K]
    msl = (-sf[:, None] * np.sin(ph)).astype(np.float32)     # [C, K]
    lam_tile = np.broadcast_to(lam.astype(np.float32)[:, None], (C, k)).copy()
    kb = k * beta
    ck = np.cos(kb).astype(np.float32)[:, None]              # [C, 1]
    sk = np.sin(kb).astype(np.float32)[:, None]              # [C, 1]
    return mcl, msl, lam_tile, ck, sk


_CACHED_NC = None


def kernel(inp, coef_re, coef_im, factor):
    global _CACHED_NC
    inp = np.ascontiguousarray(np.asarray(inp, np.float32))
    assert inp.shape == (B, T)
    mcl, msl, lam_tile, ck, sk = make_tables(coef_re, coef_im, factor)

    if _CACHED_NC is None:
        _CACHED_NC = build_bass()
    nc = _CACHED_NC

    in_maps = [
        {"x": inp[i:i + 1, :], "mcl": mcl, "msl": msl, "lamt": lam_tile,
         "ck": ck, "sk": sk}
        for i in range(B)
    ]
    res = run_bass_kernel_spmd(nc, in_maps, core_ids=list(range(B)))
    out = np.stack([np.asarray(res.results[i]["out"]).T for i in range(B)])
    return np.ascontiguousarray(out.astype(np.float32))



# revision 2
# speedup vs baseline: 5.4456x; 5.4456x over previous
"""Gammatone filterbank on TRN2 as a truncated-FIR matmul (PE engine).

The module is 4 cascaded identical complex one-pole IIR sections per band;
its exact impulse response is h_c[j] = factor_c * C(j+3,3) * lam_c^j *
cos(beta_c * j) (real part; the input is real).  |coef| <= 0.985 so h decays
geometrically: truncating at J_c taps (J_c chosen per band from the tail L2
norm, <= 768) keeps the max error ~1e-3 of output scale -- far inside the
2e-2 gate -- and turns the whole cascade into one batched FIR.

The FIR is evaluated on the Tensor engine: for each 128-sample output block
m, out[t, c] = sum_b lhsT_b^T @ taps_b where lhsT_b[p, t] = x[128(m-b) + t +
p - 127] is a 128x128 slice of a precomputed Toeplitz "strip" S[p, u] =
x_pad[u + p] (one overlapping-AP DMA builds it; bf16), and taps_b[p, c] =
h_c[128 b + 127 - p] (constant, bf16).  Bands need 1..6 tap blocks; bands
are grouped by block count so PE work is ~285 psum rows per output block
(~30 us total) instead of 6*128.  PSUM accumulates in f32; each 2 KiB bank
holds 4 output blocks, is copied (f32->bf16) to an SBUF stage by whichever
engine is free, and staged groups of 16 blocks go to DRAM with one DMA
(1 KiB+ descriptors keep the DMA bus at full model bandwidth).

Output DRAM layout is [t_local, m, c] bf16; the host transposes to
[T, C] f32 (host work is not on the device critical path).  Total DMA is
~8.5 MB strip in + ~8 MB out vs 16 MB out alone for the f32 IIR baseline,
and the serial DVE scan chain (8 scans x 32000 cols at 0.96 GHz ~ 270 us)
disappears entirely.

Sharding: batch-parallel SPMD, one waveform per NeuronCore (8 cores, B=8).
"""

import sys

import numpy as np

for _p in ("/opt/trn_rl_repo",):
    if _p not in sys.path:
        sys.path.insert(0, _p)

import ml_dtypes

import concourse.bass as bass  # noqa: F401
import concourse.mybir as mybir
from concourse.bacc import Bacc
from concourse.bass_utils import run_bass_kernel_spmd
from concourse.tile import TileContext

B = 8
T = 32000
C = 128
MB = T // 128            # 250 output blocks of 128 samples
NMAX = 6                 # max tap blocks per band (768 taps)
TOL = 1e-3               # tail L2 threshold for per-band tap count
MIN_GROUP = 8            # merge band groups smaller than this
STRIP_CHUNK = 64         # strip DMA chunk, in 128-col blocks
BANK_BLOCKS = 4          # output blocks per PSUM bank (4*128 f32 = 2 KiB)
DMA_BANKS = 4            # PSUM banks staged per output DMA
BF16 = mybir.dt.bfloat16
F32 = mybir.dt.float32
NPBF16 = ml_dtypes.bfloat16

PADB = NMAX - 1
STRIP_BLOCKS = MB + PADB            # 255
STRIP_COLS = STRIP_BLOCKS * 128     # 32640
XPAD_LEN = STRIP_COLS + 128         # 32768


def _fir_design(coef_re, coef_im, factor):
    """Exact cascade impulse response h[c, j] and its envelope, j < NMAX*128."""
    cr = np.asarray(coef_re, np.float64)
    ci = np.asarray(coef_im, np.float64)
    f = np.asarray(factor, np.float64)
    lam = np.hypot(cr, ci)
    beta = np.arctan2(ci, cr)
    j = np.arange(NMAX * 128, dtype=np.float64)
    cj = (j + 1.0) * (j + 2.0) * (j + 3.0) / 6.0
    env = f[:, None] * cj[None, :] * lam[:, None] ** j[None, :]
    h = env * np.cos(beta[:, None] * j[None, :])
    return h, env


def _plan_groups(env):
    """Per-band tap-block counts -> channel groups [(c0, c1, nblocks)]."""
    tail = np.sqrt((env ** 2)[:, ::-1].cumsum(axis=1))[:, ::-1]
    jreq = (tail > TOL).sum(axis=1)
    nblk = np.clip(np.ceil(jreq / 128.0).astype(int), 1, NMAX)
    # prefix grouping needs nblk non-increasing in c (true for this bank,
    # enforce anyway)
    nblk = np.maximum.accumulate(nblk[::-1])[::-1]
    groups = []
    c0 = 0
    for c in range(1, C + 1):
        if c == C or nblk[c] != nblk[c0]:
            groups.append([c0, c, int(nblk[c0])])
            c0 = c
    # absorb runt groups into a neighbor, keeping the larger block count
    merged = []
    for g in groups:
        if merged and (g[1] - g[0] < MIN_GROUP or merged[-1][1] - merged[-1][0] < MIN_GROUP):
            merged[-1][1] = g[1]
        else:
            merged.append(g)
    return [tuple(g) for g in merged], nblk


def build_bass(groups):
    nc = Bacc()
    xp = nc.declare_dram_parameter("xp", [1, XPAD_LEN], BF16, isOutput=False)
    tp = nc.declare_dram_parameter("taps", [128, NMAX * 128], BF16, isOutput=False)
    out = nc.declare_dram_parameter("out", [128, MB, C], BF16, isOutput=True)

    with TileContext(nc) as tc:
        with (
            tc.tile_pool(name="consts", bufs=1) as consts,
            tc.tile_pool(name="psum", bufs=6, space="PSUM") as psum_pool,
            tc.tile_pool(name="stage", bufs=3) as stage_pool,
        ):
            taps = consts.tile([128, NMAX * 128], BF16, tag="taps", name="taps")
            nc.sync.dma_start(out=taps[:], in_=tp[:, :])

            strips = []
            for i in range(0, STRIP_BLOCKS, STRIP_CHUNK):
                nb = min(STRIP_CHUNK, STRIP_BLOCKS - i)
                st = consts.tile([128, nb * 128], BF16, tag=f"strip{i}",
                                 name=f"strip{i}")
                src = bass.AP(xp, i * 128, [[1, 128], [1, nb * 128]])
                nc.sync.dma_start(out=st[:], in_=src)
                strips.append(st)

            def strip_slice(k):
                """[128, 128] lhsT view for strip block k (0..STRIP_BLOCKS-1)."""
                ti, tb = divmod(k, STRIP_CHUNK)
                return strips[ti][:, tb * 128:(tb + 1) * 128]

            dma_blocks = DMA_BANKS * BANK_BLOCKS
            for dg in range(0, MB, dma_blocks):
                mg = min(dma_blocks, MB - dg)
                staged = stage_pool.tile([128, mg, C], BF16, tag="staged",
                                         name="staged")
                for bq in range(0, mg, BANK_BLOCKS):
                    nb = min(BANK_BLOCKS, mg - bq)
                    pt = psum_pool.tile([128, nb, C], F32, tag="bank", name="pt")
                    for ms in range(nb):
                        m = dg + bq + ms
                        for (c0, c1, ng) in groups:
                            for b in range(ng):
                                nc.tensor.matmul(
                                    pt[:, ms, c0:c1],
                                    lhsT=strip_slice(m - b + PADB),
                                    rhs=taps[:, 128 * b + c0:128 * b + c1],
                                    start=(b == 0),
                                    stop=(b == ng - 1),
                                )
                    nc.any.tensor_copy(staged[:, bq:bq + nb, :], pt[:, :, :])
                nc.sync.dma_start(out=out[:, dg:dg + mg, :], in_=staged[:, :, :])
    nc.finalize()
    return nc


def make_tables(coef_re, coef_im, factor):
    h, env = _fir_design(coef_re, coef_im, factor)
    groups, nblk = _plan_groups(env)
    nper = np.empty(C, int)
    for c0, c1, ng in groups:
        nper[c0:c1] = ng
    hz = h.copy()
    for c in range(C):
        hz[c, nper[c] * 128:] = 0.0
    # tapsT[p, 128*b + c] = hz[c, 128*b + 127 - p]
    hb = hz.reshape(C, NMAX, 128)          # [c, b, j0]
    tapsT = hb[:, :, ::-1].transpose(2, 1, 0).reshape(128, NMAX * C)
    return np.ascontiguousarray(tapsT.astype(NPBF16)), groups


_CACHE = {}


def kernel(inp, coef_re, coef_im, factor):
    inp = np.ascontiguousarray(np.asarray(inp, np.float32))
    assert inp.shape == (B, T)
    tapsT, groups = make_tables(coef_re, coef_im, factor)

    key = tuple(groups)
    if key not in _CACHE:
        _CACHE[key] = build_bass(groups)
    nc = _CACHE[key]

    xpad = np.zeros((B, XPAD_LEN), np.float32)
    xpad[:, 767:767 + T] = inp
    xpad = xpad.astype(NPBF16)

    in_maps = [
        {"xp": xpad[i:i + 1], "taps": tapsT}
        for i in range(B)
    ]
    res = run_bass_kernel_spmd(nc, in_maps, core_ids=list(range(B)))
    out = np.stack([
        np.asarray(res.results[i]["out"]).astype(np.float32)
        .transpose(1, 0, 2).reshape(T, C)
        for i in range(B)
    ])
    return np.ascontiguousarray(out)


# revision 5
# speedup vs baseline: 5.8741x; 1.0787x over previous
"""Gammatone filterbank on TRN2 as a truncated-FIR matmul (PE engine).

The module is 4 cascaded identical complex one-pole IIR sections per band;
its exact impulse response is h_c[j] = factor_c * C(j+3,3) * lam_c^j *
cos(beta_c * j) (real part; the input is real).  |coef| <= 0.985 so h decays
geometrically: truncating at J_c taps (J_c chosen per band from the tail L2
norm, <= 768) keeps the max error ~1e-3 of output scale -- far inside the
2e-2 gate -- and turns the whole cascade into one batched FIR.

The FIR is evaluated on the Tensor engine: for each 128-sample output block
m, out[t, c] = sum_b lhsT_b^T @ taps_b where lhsT_b[p, t] = x[128(m-b) + t +
p - 127] is a 128x128 slice of a precomputed Toeplitz "strip" S[p, u] =
x_pad[u + p] (one overlapping-AP DMA builds it; bf16), and taps_b[p, c] =
h_c[128 b + 127 - p] (constant, bf16).  Bands need 1..6 tap blocks; bands
are grouped by block count so PE work is ~285 psum rows per output block
(~30 us total) instead of 6*128.  PSUM accumulates in f32; each 2 KiB bank
holds 4 output blocks, is copied (f32->bf16) to an SBUF stage by whichever
engine is free, and staged groups of 16 blocks go to DRAM with one DMA
(1 KiB+ descriptors keep the DMA bus at full model bandwidth).

Output DRAM layout is [t_local, m, c] bf16; the host transposes to
[T, C] f32 (host work is not on the device critical path).  Total DMA is
~8.5 MB strip in + ~8 MB out vs 16 MB out alone for the f32 IIR baseline,
and the serial DVE scan chain (8 scans x 32000 cols at 0.96 GHz ~ 270 us)
disappears entirely.

Sharding: batch-parallel SPMD, one waveform per NeuronCore (8 cores, B=8).
"""

import sys

import numpy as np

for _p in ("/opt/trn_rl_repo",):
    if _p not in sys.path:
        sys.path.insert(0, _p)

import ml_dtypes

import concourse.bass as bass  # noqa: F401
import concourse.mybir as mybir
from concourse.bacc import Bacc
from concourse.bass_utils import run_bass_kernel_spmd
from concourse.tile import TileContext

B = 8
T = 32000
C = 128
MB = T // 128            # 250 output blocks of 128 samples
NMAX = 6                 # max tap blocks per band (768 taps)
TOL = 1e-3               # tail L2 threshold for per-band tap count
MIN_GROUP = 8            # merge band groups smaller than this
STRIP_CHUNK = 64         # strip DMA chunk, in 128-col blocks
BANK_BLOCKS = 2          # output blocks per PSUM accumulation tile
DMA_BANKS = 8            # PSUM tiles staged per output DMA (16 blocks)
BF16 = mybir.dt.bfloat16
F32 = mybir.dt.float32
NPBF16 = ml_dtypes.bfloat16

PADB = NMAX - 1
STRIP_BLOCKS = MB + PADB            # 255
STRIP_COLS = STRIP_BLOCKS * 128     # 32640
XPAD_LEN = STRIP_COLS + 128         # 32768


def _fir_design(coef_re, coef_im, factor):
    """Exact cascade impulse response h[c, j] and its envelope, j < NMAX*128."""
    cr = np.asarray(coef_re, np.float64)
    ci = np.asarray(coef_im, np.float64)
    f = np.asarray(factor, np.float64)
    lam = np.hypot(cr, ci)
    beta = np.arctan2(ci, cr)
    j = np.arange(NMAX * 128, dtype=np.float64)
    cj = (j + 1.0) * (j + 2.0) * (j + 3.0) / 6.0
    env = f[:, None] * cj[None, :] * lam[:, None] ** j[None, :]
    h = env * np.cos(beta[:, None] * j[None, :])
    return h, env


def _plan_groups(env):
    """Per-band tap-block counts -> channel groups [(c0, c1, nblocks)]."""
    tail = np.sqrt((env ** 2)[:, ::-1].cumsum(axis=1))[:, ::-1]
    jreq = (tail > TOL).sum(axis=1)
    nblk = np.clip(np.ceil(jreq / 128.0).astype(int), 1, NMAX)
    # prefix grouping needs nblk non-increasing in c (true for this bank,
    # enforce anyway)
    nblk = np.maximum.accumulate(nblk[::-1])[::-1]
    groups = []
    c0 = 0
    for c in range(1, C + 1):
        if c == C or nblk[c] != nblk[c0]:
            groups.append([c0, c, int(nblk[c0])])
            c0 = c
    # absorb runt groups into a neighbor, keeping the larger block count
    merged = []
    for g in groups:
        if merged and (g[1] - g[0] < MIN_GROUP or merged[-1][1] - merged[-1][0] < MIN_GROUP):
            merged[-1][1] = g[1]
        else:
            merged.append(g)
    return [tuple(g) for g in merged], nblk


def build_bass(groups):
    nc = Bacc()
    xp = nc.declare_dram_parameter("xp", [1, XPAD_LEN], BF16, isOutput=False)
    tp = nc.declare_dram_parameter("taps", [128, NMAX * 128], BF16, isOutput=False)
    out = nc.declare_dram_parameter("out", [128, MB, C], BF16, isOutput=True)

    with TileContext(nc) as tc:
        with (
            tc.tile_pool(name="consts", bufs=1) as consts,
            tc.tile_pool(name="psum", bufs=8, space="PSUM") as psum_pool,
            tc.tile_pool(name="stage", bufs=8) as stage_pool,
        ):
            taps = consts.tile([128, NMAX * 128], BF16, tag="taps", name="taps")
            strips = []
            strip_dmas = []
            for i in range(0, STRIP_BLOCKS, STRIP_CHUNK):
                nb = min(STRIP_CHUNK, STRIP_BLOCKS - i)
                st = consts.tile([128, nb * 128], BF16, tag=f"strip{i}",
                                 name=f"strip{i}")
                src = bass.AP(xp, i * 128, [[1, 128], [1, nb * 128]])
                strips.append(st)
                strip_dmas.append((st, src))
            # first strip chunk before the taps so PE's first dependency
            # lands as early as possible
            nc.sync.dma_start(out=strip_dmas[0][0][:], in_=strip_dmas[0][1])
            nc.sync.dma_start(out=taps[:], in_=tp[:, :])
            for st, src in strip_dmas[1:]:
                nc.sync.dma_start(out=st[:], in_=src)

            def strip_slice(k):
                """[128, 128] lhsT view for strip block k (0..STRIP_BLOCKS-1)."""
                ti, tb = divmod(k, STRIP_CHUNK)
                return strips[ti][:, tb * 128:(tb + 1) * 128]

            dma_blocks = DMA_BANKS * BANK_BLOCKS
            for dg in range(0, MB, dma_blocks):
                mg = min(dma_blocks, MB - dg)
                staged = stage_pool.tile([128, mg, C], BF16, tag="staged",
                                         name="staged")
                for bq in range(0, mg, BANK_BLOCKS):
                    nb = min(BANK_BLOCKS, mg - bq)
                    pt = psum_pool.tile([128, nb, C], F32, tag="bank", name="pt")
                    for ms in range(nb):
                        m = dg + bq + ms
                        for (c0, c1, ng) in groups:
                            for b in range(ng):
                                nc.tensor.matmul(
                                    pt[:, ms, c0:c1],
                                    lhsT=strip_slice(m - b + PADB),
                                    rhs=taps[:, 128 * b + c0:128 * b + c1],
                                    start=(b == 0),
                                    stop=(b == ng - 1),
                                )
                    nc.any.tensor_copy(staged[:, bq:bq + nb, :], pt[:, :, :])
                nc.sync.dma_start(out=out[:, dg:dg + mg, :], in_=staged[:, :, :])
    nc.finalize()
    return nc


def make_tables(coef_re, coef_im, factor):
    h, env = _fir_design(coef_re, coef_im, factor)
    groups, nblk = _plan_groups(env)
    nper = np.empty(C, int)
    for c0, c1, ng in groups:
        nper[c0:c1] = ng
    hz = h.copy()
    for c in range(C):
        hz[c, nper[c] * 128:] = 0.0
    # tapsT[p, 128*b + c] = hz[c, 128*b + 127 - p]
    hb = hz.reshape(C, NMAX, 128)          # [c, b, j0]
    tapsT = hb[:, :, ::-1].transpose(2, 1, 0).reshape(128, NMAX * C)
    return np.ascontiguousarray(tapsT.astype(NPBF16)), groups


_CACHE = {}


def kernel(inp, coef_re, coef_im, factor):
    inp = np.ascontiguousarray(np.asarray(inp, np.float32))
    assert inp.shape == (B, T)
    tapsT, groups = make_tables(coef_re, coef_im, factor)

    key = tuple(groups)
    if key not in _CACHE:
        _CACHE[key] = build_bass(groups)
    nc = _CACHE[key]

    xpad = np.zeros((B, XPAD_LEN), np.float32)
    xpad[:, 767:767 + T] = inp
    xpad = xpad.astype(NPBF16)

    in_maps = [
        {"xp": xpad[i:i + 1], "taps": tapsT}
        for i in range(B)
    ]
    res = run_bass_kernel_spmd(nc, in_maps, core_ids=list(range(B)))
    out = np.stack([
        np.asarray(res.results[i]["out"]).astype(np.float32)
        .transpose(1, 0, 2).reshape(T, C)
        for i in range(B)
    ])
    return np.ascontiguousarray(out)


# revision 9
# speedup vs baseline: 5.9406x; 1.0113x over previous
"""Gammatone filterbank on TRN2 as a truncated-FIR matmul (PE engine).

The module is 4 cascaded identical complex one-pole IIR sections per band;
its exact impulse response is h_c[j] = factor_c * C(j+3,3) * lam_c^j *
cos(beta_c * j) (real part; the input is real).  |coef| <= 0.985 so h decays
geometrically: truncating at J_c taps (J_c chosen per band from the tail L2
norm, <= 768) keeps the max error ~1e-3 of output scale -- far inside the
2e-2 gate -- and turns the whole cascade into one batched FIR.

The FIR is evaluated on the Tensor engine: for each 128-sample output block
m, out[t, c] = sum_b lhsT_b^T @ taps_b where lhsT_b[p, t] = x[128(m-b) + t +
p - 127] is a 128x128 slice of a precomputed Toeplitz "strip" S[p, u] =
x_pad[u + p] (one overlapping-AP DMA builds it; bf16), and taps_b[p, c] =
h_c[128 b + 127 - p] (constant, bf16).  Bands need 1..6 tap blocks; bands
are grouped by block count so PE work is ~285 psum rows per output block
(~30 us total) instead of 6*128.  PSUM accumulates in f32; each 2 KiB bank
holds 4 output blocks, is copied (f32->bf16) to an SBUF stage by whichever
engine is free, and staged groups of 16 blocks go to DRAM with one DMA
(1 KiB+ descriptors keep the DMA bus at full model bandwidth).

Output DRAM layout is [t_local, m, c] bf16; the host transposes to
[T, C] f32 (host work is not on the device critical path).  Total DMA is
~8.5 MB strip in + ~8 MB out vs 16 MB out alone for the f32 IIR baseline,
and the serial DVE scan chain (8 scans x 32000 cols at 0.96 GHz ~ 270 us)
disappears entirely.

Sharding: batch-parallel SPMD, one waveform per NeuronCore (8 cores, B=8).
"""

import sys

import numpy as np

for _p in ("/opt/trn_rl_repo",):
    if _p not in sys.path:
        sys.path.insert(0, _p)

import ml_dtypes

import concourse.bass as bass  # noqa: F401
import concourse.mybir as mybir
from concourse.bacc import Bacc
from concourse.bass_utils import run_bass_kernel_spmd
from concourse.tile import TileContext

B = 8
T = 32000
C = 128
MB = T // 128            # 250 output blocks of 128 samples
NMAX = 6                 # max tap blocks per band (768 taps)
TOL = 1e-3               # tail L2 threshold for per-band tap count
MIN_GROUP = 8            # merge band groups smaller than this
STRIP_CHUNK = 64         # strip DMA chunk, in 128-col blocks
BANK_BLOCKS = 2          # output blocks per PSUM accumulation tile
DMA_BLOCKS = 32          # output blocks staged per output DMA
STAGE_BUFS = 6
STRIP_CHUNKS = (32, 80, 80, 63)   # strip DMA chunk sizes, in 128-col blocks
BF16 = mybir.dt.bfloat16
F32 = mybir.dt.float32
NPBF16 = ml_dtypes.bfloat16

PADB = NMAX - 1
STRIP_BLOCKS = MB + PADB            # 255
STRIP_COLS = STRIP_BLOCKS * 128     # 32640
XPAD_LEN = STRIP_COLS + 128         # 32768


def _fir_design(coef_re, coef_im, factor):
    """Exact cascade impulse response h[c, j] and its envelope, j < NMAX*128."""
    cr = np.asarray(coef_re, np.float64)
    ci = np.asarray(coef_im, np.float64)
    f = np.asarray(factor, np.float64)
    lam = np.hypot(cr, ci)
    beta = np.arctan2(ci, cr)
    j = np.arange(NMAX * 128, dtype=np.float64)
    cj = (j + 1.0) * (j + 2.0) * (j + 3.0) / 6.0
    env = f[:, None] * cj[None, :] * lam[:, None] ** j[None, :]
    h = env * np.cos(beta[:, None] * j[None, :])
    return h, env


def _plan_groups(env):
    """Per-band tap-block counts -> channel groups [(c0, c1, nblocks)]."""
    tail = np.sqrt((env ** 2)[:, ::-1].cumsum(axis=1))[:, ::-1]
    jreq = (tail > TOL).sum(axis=1)
    nblk = np.clip(np.ceil(jreq / 128.0).astype(int), 1, NMAX)
    # prefix grouping needs nblk non-increasing in c (true for this bank,
    # enforce anyway)
    nblk = np.maximum.accumulate(nblk[::-1])[::-1]
    groups = []
    c0 = 0
    for c in range(1, C + 1):
        if c == C or nblk[c] != nblk[c0]:
            groups.append([c0, c, int(nblk[c0])])
            c0 = c
    # absorb runt groups into a neighbor, keeping the larger block count
    merged = []
    for g in groups:
        if merged and (g[1] - g[0] < MIN_GROUP or merged[-1][1] - merged[-1][0] < MIN_GROUP):
            merged[-1][1] = g[1]
        else:
            merged.append(g)
    return [tuple(g) for g in merged], nblk


def build_bass(groups):
    nc = Bacc()
    xp = nc.declare_dram_parameter("xp", [1, XPAD_LEN], BF16, isOutput=False)
    tp = nc.declare_dram_parameter("taps", [128, NMAX * 128], BF16, isOutput=False)
    out = nc.declare_dram_parameter("out", [128, MB, C], BF16, isOutput=True)

    with TileContext(nc) as tc:
        with (
            tc.tile_pool(name="consts", bufs=1) as consts,
            tc.tile_pool(name="psum", bufs=8, space="PSUM") as psum_pool,
            tc.tile_pool(name="stage", bufs=STAGE_BUFS) as stage_pool,
        ):
            taps = consts.tile([128, NMAX * 128], BF16, tag="taps", name="taps")
            # taps DMA issued from the Activation engine so its HWDGE/SEQ
            # work stays off the SP chain that feeds the strip
            nc.scalar.dma_start(out=taps[:], in_=tp[:, :])

            strips = []
            chunk_start = []
            i = 0
            for nb in STRIP_CHUNKS:
                st = consts.tile([128, nb * 128], BF16, tag=f"strip{i}",
                                 name=f"strip{i}")
                src = bass.AP(xp, i * 128, [[1, 128], [1, nb * 128]])
                nc.sync.dma_start(out=st[:], in_=src)
                strips.append(st)
                chunk_start.append(i)
                i += nb
            assert i == STRIP_BLOCKS

            def strip_slice(k):
                """[128, 128] lhsT view for strip block k (0..STRIP_BLOCKS-1)."""
                for ti in range(len(strips) - 1, -1, -1):
                    if k >= chunk_start[ti]:
                        tb = k - chunk_start[ti]
                        return strips[ti][:, tb * 128:(tb + 1) * 128]
                raise AssertionError(k)

            for dg in range(0, MB, DMA_BLOCKS):
                mg = min(DMA_BLOCKS, MB - dg)
                staged = stage_pool.tile([128, mg, C], BF16, tag="staged",
                                         name="staged")
                for bq in range(0, mg, BANK_BLOCKS):
                    nb = min(BANK_BLOCKS, mg - bq)
                    pt = psum_pool.tile([128, nb, C], F32, tag="bank", name="pt")
                    for ms in range(nb):
                        m = dg + bq + ms
                        for (c0, c1, ng) in groups:
                            for b in range(ng):
                                nc.tensor.matmul(
                                    pt[:, ms, c0:c1],
                                    lhsT=strip_slice(m - b + PADB),
                                    rhs=taps[:, 128 * b + c0:128 * b + c1],
                                    start=(b == 0),
                                    stop=(b == ng - 1),
                                )
                    nc.any.tensor_copy(staged[:, bq:bq + nb, :], pt[:, :, :])
                nc.sync.dma_start(out=out[:, dg:dg + mg, :], in_=staged[:, :, :])
    nc.finalize()
    return nc


def make_tables(coef_re, coef_im, factor):
    h, env = _fir_design(coef_re, coef_im, factor)
    groups, nblk = _plan_groups(env)
    nper = np.empty(C, int)
    for c0, c1, ng in groups:
        nper[c0:c1] = ng
    hz = h.copy()
    for c in range(C):
        hz[c, nper[c] * 128:] = 0.0
    # tapsT[p, 128*b + c] = hz[c, 128*b + 127 - p]
    hb = hz.reshape(C, NMAX, 128)          # [c, b, j0]
    tapsT = hb[:, :, ::-1].transpose(2, 1, 0).reshape(128, NMAX * C)
    return np.ascontiguousarray(tapsT.astype(NPBF16)), groups


_CACHE = {}


def kernel(inp, coef_re, coef_im, factor):
    inp = np.ascontiguousarray(np.asarray(inp, np.float32))
    assert inp.shape == (B, T)
    tapsT, groups = make_tables(coef_re, coef_im, factor)

    key = tuple(groups)
    if key not in _CACHE:
        _CACHE[key] = build_bass(groups)
    nc = _CACHE[key]

    xpad = np.zeros((B, XPAD_LEN), np.float32)
    xpad[:, 767:767 + T] = inp
    xpad = xpad.astype(NPBF16)

    in_maps = [
        {"xp": xpad[i:i + 1], "taps": tapsT}
        for i in range(B)
    ]
    res = run_bass_kernel_spmd(nc, in_maps, core_ids=list(range(B)))
    out = np.stack([
        np.asarray(res.results[i]["out"]).astype(np.float32)
        .transpose(1, 0, 2).reshape(T, C)
        for i in range(B)
    ])
    return np.ascontiguousarray(out)


# revision 15
# speedup vs baseline: 5.9477x; 1.0012x over previous
"""Gammatone filterbank on TRN2 as a truncated-FIR matmul (PE engine).

The module is 4 cascaded identical complex one-pole IIR sections per band;
its exact impulse response is h_c[j] = factor_c * C(j+3,3) * lam_c^j *
cos(beta_c * j) (real part; the input is real).  |coef| <= 0.985 so h decays
geometrically: truncating at J_c taps (J_c chosen per band from the tail L2
norm, <= 768) keeps the max error ~1e-3 of output scale -- far inside the
2e-2 gate -- and turns the whole cascade into one batched FIR.

The FIR is evaluated on the Tensor engine: for each 128-sample output block
m, out[t, c] = sum_b lhsT_b^T @ taps_b where lhsT_b[p, t] = x[128(m-b) + t +
p - 127] is a 128x128 slice of a precomputed Toeplitz "strip" S[p, u] =
x_pad[u + p] (one overlapping-AP DMA builds it; bf16), and taps_b[p, c] =
h_c[128 b + 127 - p] (constant, bf16).  Bands need 1..6 tap blocks; bands
are grouped by block count so PE work is ~285 psum rows per output block
(~30 us total) instead of 6*128.  PSUM accumulates in f32; each 2 KiB bank
holds 4 output blocks, is copied (f32->bf16) to an SBUF stage by whichever
engine is free, and staged groups of 16 blocks go to DRAM with one DMA
(1 KiB+ descriptors keep the DMA bus at full model bandwidth).

Output DRAM layout is [t_local, m, c] bf16; the host transposes to
[T, C] f32 (host work is not on the device critical path).  Total DMA is
~8.5 MB strip in + ~8 MB out vs 16 MB out alone for the f32 IIR baseline,
and the serial DVE scan chain (8 scans x 32000 cols at 0.96 GHz ~ 270 us)
disappears entirely.

Sharding: batch-parallel SPMD, one waveform per NeuronCore (8 cores, B=8).
"""

import sys

import numpy as np

for _p in ("/opt/trn_rl_repo",):
    if _p not in sys.path:
        sys.path.insert(0, _p)

import ml_dtypes

import concourse.bass as bass  # noqa: F401
import concourse.mybir as mybir
from concourse.bacc import Bacc
from concourse.bass_utils import run_bass_kernel_spmd
from concourse.tile import TileContext

B = 8
T = 32000
C = 128
MB = T // 128            # 250 output blocks of 128 samples
KTAP = 112               # taps per matmul contraction block (<= 128)
NMAX = 6                 # max tap blocks per band
TOL = 2e-3               # tail L2 threshold for per-band tap count
MIN_GROUP = 8            # merge band groups smaller than this
BANK_BLOCKS = 2          # output blocks per PSUM accumulation tile
DMA_BLOCKS = 32          # output blocks staged per output DMA
STAGE_BUFS = 6
N_STRIP_DMAS = 4
BF16 = mybir.dt.bfloat16
F32 = mybir.dt.float32
NPBF16 = ml_dtypes.bfloat16

OFF0 = (NMAX - 1) * KTAP            # strip column of (m=0, b=NMAX-1) window
STRIP_COLS = 128 * (MB - 1) + 128 + OFF0
XPAD_OFF = OFF0 + KTAP - 1          # leading zeros in x_pad
XPAD_LEN = STRIP_COLS + KTAP        # >= STRIP_COLS-1 + (KTAP-1) + 1


def _fir_design(coef_re, coef_im, factor):
    """Exact cascade impulse response h[c, j] and its envelope, j < NMAX*KTAP."""
    cr = np.asarray(coef_re, np.float64)
    ci = np.asarray(coef_im, np.float64)
    f = np.asarray(factor, np.float64)
    lam = np.hypot(cr, ci)
    beta = np.arctan2(ci, cr)
    j = np.arange(NMAX * KTAP, dtype=np.float64)
    cj = (j + 1.0) * (j + 2.0) * (j + 3.0) / 6.0
    env = f[:, None] * cj[None, :] * lam[:, None] ** j[None, :]
    h = env * np.cos(beta[:, None] * j[None, :])
    return h, env


def _plan_groups(env):
    """Per-band tap-block counts -> channel groups [(c0, c1, nblocks)]."""
    tail = np.sqrt((env ** 2)[:, ::-1].cumsum(axis=1))[:, ::-1]
    jreq = (tail > TOL).sum(axis=1)
    nblk = np.clip(np.ceil(jreq / float(KTAP)).astype(int), 1, NMAX)
    # prefix grouping needs nblk non-increasing in c (true for this bank,
    # enforce anyway)
    nblk = np.maximum.accumulate(nblk[::-1])[::-1]
    groups = []
    c0 = 0
    for c in range(1, C + 1):
        if c == C or nblk[c] != nblk[c0]:
            groups.append([c0, c, int(nblk[c0])])
            c0 = c
    # absorb runt groups into a neighbor, keeping the larger block count
    merged = []
    for g in groups:
        if merged and (g[1] - g[0] < MIN_GROUP or merged[-1][1] - merged[-1][0] < MIN_GROUP):
            merged[-1][1] = g[1]
        else:
            merged.append(g)
    return [tuple(g) for g in merged], nblk


def build_bass(groups):
    nc = Bacc()
    xp = nc.declare_dram_parameter("xp", [1, XPAD_LEN], BF16, isOutput=False)
    tp = nc.declare_dram_parameter("taps", [KTAP, NMAX * 128], BF16,
                                   isOutput=False)
    out = nc.declare_dram_parameter("out", [128, MB, C], BF16, isOutput=True)

    with TileContext(nc) as tc:
        with (
            tc.tile_pool(name="consts", bufs=1) as consts,
            tc.tile_pool(name="psum", bufs=8, space="PSUM") as psum_pool,
            tc.tile_pool(name="stage", bufs=STAGE_BUFS) as stage_pool,
        ):
            taps = consts.tile([KTAP, NMAX * 128], BF16, tag="taps",
                               name="taps")
            # taps DMA issued from the Activation engine so its HWDGE/SEQ
            # work stays off the SP chain that feeds the strip
            nc.scalar.dma_start(out=taps[:], in_=tp[:, :])

            # one Toeplitz strip tile, filled by a few column-range DMAs
            # (subtile deps let matmuls start once their window has landed);
            # the first range is small so PE's first dependency lands early
            strip = consts.tile([KTAP, STRIP_COLS], BF16, tag="strip",
                                name="strip")
            bounds = [0, 4096]
            rest = STRIP_COLS - 4096
            for i in range(N_STRIP_DMAS - 1):
                bounds.append(bounds[-1] + rest // (N_STRIP_DMAS - 1))
            bounds[-1] = STRIP_COLS
            for a, bnd in zip(bounds[:-1], bounds[1:]):
                src = bass.AP(xp, a, [[1, KTAP], [1, bnd - a]])
                nc.sync.dma_start(out=strip[:, a:bnd], in_=src)

            for dg in range(0, MB, DMA_BLOCKS):
                mg = min(DMA_BLOCKS, MB - dg)
                staged = stage_pool.tile([128, mg, C], BF16, tag="staged",
                                         name="staged")
                for bq in range(0, mg, BANK_BLOCKS):
                    nb = min(BANK_BLOCKS, mg - bq)
                    pt = psum_pool.tile([128, nb, C], F32, tag="bank", name="pt")
                    for ms in range(nb):
                        m = dg + bq + ms
                        for (c0, c1, ng) in groups:
                            for b in range(ng):
                                u0 = 128 * m - KTAP * b + OFF0
                                nc.tensor.matmul(
                                    pt[:, ms, c0:c1],
                                    lhsT=strip[:, u0:u0 + 128],
                                    rhs=taps[:, 128 * b + c0:128 * b + c1],
                                    start=(b == 0),
                                    stop=(b == ng - 1),
                                )
                    nc.any.tensor_copy(staged[:, bq:bq + nb, :], pt[:, :, :])
                nc.sync.dma_start(out=out[:, dg:dg + mg, :], in_=staged[:, :, :])
    nc.finalize()
    return nc


def make_tables(coef_re, coef_im, factor):
    h, env = _fir_design(coef_re, coef_im, factor)
    groups, nblk = _plan_groups(env)
    nper = np.empty(C, int)
    for c0, c1, ng in groups:
        nper[c0:c1] = ng
    hz = h.copy()
    for c in range(C):
        hz[c, nper[c] * KTAP:] = 0.0
    # tapsT[p, 128*b + c] = hz[c, KTAP*b + (KTAP-1) - p]
    hb = hz.reshape(C, NMAX, KTAP)         # [c, b, j0]
    tapsT = hb[:, :, ::-1].transpose(2, 1, 0).reshape(KTAP, NMAX * C)
    return np.ascontiguousarray(tapsT.astype(NPBF16)), groups


_CACHE = {}


def kernel(inp, coef_re, coef_im, factor):
    inp = np.ascontiguousarray(np.asarray(inp, np.float32))
    assert inp.shape == (B, T)
    tapsT, groups = make_tables(coef_re, coef_im, factor)

    key = tuple(groups)
    if key not in _CACHE:
        _CACHE[key] = build_bass(groups)
    nc = _CACHE[key]

    xpad = np.zeros((B, XPAD_LEN), np.float32)
    xpad[:, XPAD_OFF:XPAD_OFF + T] = inp
    xpad = xpad.astype(NPBF16)

    in_maps = [
        {"xp": xpad[i:i + 1], "taps": tapsT}
        for i in range(B)
    ]
    res = run_bass_kernel_spmd(nc, in_maps, core_ids=list(range(B)))
    out = np.stack([
        np.asarray(res.results[i]["out"]).astype(np.float32)
        .transpose(1, 0, 2).reshape(T, C)
        for i in range(B)
    ])
    return np.ascontiguousarray(out)


# revision 19
# speedup vs baseline: 6.5608x; 1.1031x over previous
"""Gammatone filterbank on TRN2 as a truncated-FIR matmul (PE engine).

The module is 4 cascaded identical complex one-pole IIR sections per band;
its exact impulse response is h_c[j] = factor_c * C(j+3,3) * lam_c^j *
cos(beta_c * j) (real part; the input is real).  |coef| <= 0.985 so h decays
geometrically: truncating at J_c taps (J_c chosen per band from the tail L2
norm, <= 768) keeps the max error ~1e-3 of output scale -- far inside the
2e-2 gate -- and turns the whole cascade into one batched FIR.

The FIR is evaluated on the Tensor engine: for each 128-sample output block
m, out[t, c] = sum_b lhsT_b^T @ taps_b where lhsT_b[p, t] = x[128(m-b) + t +
p - 127] is a 128x128 slice of a precomputed Toeplitz "strip" S[p, u] =
x_pad[u + p] (one overlapping-AP DMA builds it; bf16), and taps_b[p, c] =
h_c[128 b + 127 - p] (constant, bf16).  Bands need 1..6 tap blocks; bands
are grouped by block count so PE work is ~285 psum rows per output block
(~30 us total) instead of 6*128.  PSUM accumulates in f32; each 2 KiB bank
holds 4 output blocks, is copied (f32->bf16) to an SBUF stage by whichever
engine is free, and staged groups of 16 blocks go to DRAM with one DMA
(1 KiB+ descriptors keep the DMA bus at full model bandwidth).

Output DRAM layout is [t_local, m, c] bf16; the host transposes to
[T, C] f32 (host work is not on the device critical path).  Total DMA is
~8.5 MB strip in + ~8 MB out vs 16 MB out alone for the f32 IIR baseline,
and the serial DVE scan chain (8 scans x 32000 cols at 0.96 GHz ~ 270 us)
disappears entirely.

Sharding: batch-parallel SPMD, one waveform per NeuronCore (8 cores, B=8).
"""

import sys

import numpy as np

for _p in ("/opt/trn_rl_repo",):
    if _p not in sys.path:
        sys.path.insert(0, _p)

import ml_dtypes

import concourse.bass as bass  # noqa: F401
import concourse.mybir as mybir
from concourse.bacc import Bacc
from concourse.bass_utils import run_bass_kernel_spmd
from concourse.tile import TileContext

B = 8
T = 32000
C = 128
MB = T // 128            # 250 output blocks of 128 samples
KTAP = 128               # taps per matmul contraction block (<= 128)
NMAX = 6                 # max tap blocks per band
TOL = 3e-3               # tail L2 threshold for per-band tap count
MIN_GROUP = 8            # merge band groups smaller than this
BANK_BLOCKS = 4          # output blocks per PSUM accumulation tile
DMA_BLOCKS = 32          # output blocks staged per output DMA
STAGE_BUFS = 6
BF16 = mybir.dt.bfloat16
F32 = mybir.dt.float32
NPBF16 = ml_dtypes.bfloat16

OFF0 = (NMAX - 1) * KTAP            # strip column of (m=0, b=NMAX-1) window
STRIP_COLS = 128 * (MB - 1) + 128 + OFF0
XPAD_OFF = OFF0 + KTAP - 1          # leading zeros in x_pad
XPAD_LEN = STRIP_COLS + KTAP        # >= STRIP_COLS-1 + (KTAP-1) + 1


def _fir_design(coef_re, coef_im, factor):
    """Exact cascade impulse response h[c, j] and its envelope, j < NMAX*KTAP."""
    cr = np.asarray(coef_re, np.float64)
    ci = np.asarray(coef_im, np.float64)
    f = np.asarray(factor, np.float64)
    lam = np.hypot(cr, ci)
    beta = np.arctan2(ci, cr)
    j = np.arange(NMAX * KTAP, dtype=np.float64)
    cj = (j + 1.0) * (j + 2.0) * (j + 3.0) / 6.0
    env = f[:, None] * cj[None, :] * lam[:, None] ** j[None, :]
    h = env * np.cos(beta[:, None] * j[None, :])
    return h, env


def _plan_groups(env):
    """Per-band tap-block counts -> channel groups [(c0, c1, nblocks)]."""
    tail = np.sqrt((env ** 2)[:, ::-1].cumsum(axis=1))[:, ::-1]
    jreq = (tail > TOL).sum(axis=1)
    nblk = np.clip(np.ceil(jreq / float(KTAP)).astype(int), 1, NMAX)
    # prefix grouping needs nblk non-increasing in c (true for this bank,
    # enforce anyway)
    nblk = np.maximum.accumulate(nblk[::-1])[::-1]
    groups = []
    c0 = 0
    for c in range(1, C + 1):
        if c == C or nblk[c] != nblk[c0]:
            groups.append([c0, c, int(nblk[c0])])
            c0 = c
    # absorb runt groups into a neighbor, keeping the larger block count
    merged = []
    for g in groups:
        if merged and (g[1] - g[0] < MIN_GROUP or merged[-1][1] - merged[-1][0] < MIN_GROUP):
            merged[-1][1] = g[1]
        else:
            merged.append(g)
    return [tuple(g) for g in merged], nblk


def build_bass(groups):
    nc = Bacc()
    xp = nc.declare_dram_parameter("xp", [1, XPAD_LEN], BF16, isOutput=False)
    tp = nc.declare_dram_parameter("taps", [KTAP, NMAX * 128], BF16,
                                   isOutput=False)
    out = nc.declare_dram_parameter("out", [128, MB, C], BF16, isOutput=True)

    with TileContext(nc) as tc:
        with (
            tc.tile_pool(name="consts", bufs=1) as consts,
            tc.tile_pool(name="psum", bufs=8, space="PSUM") as psum_pool,
            tc.tile_pool(name="stage", bufs=STAGE_BUFS) as stage_pool,
        ):
            taps = consts.tile([KTAP, NMAX * 128], BF16, tag="taps",
                               name="taps")
            # taps DMA issued from the Activation engine so its HWDGE/SEQ
            # work stays off the SP chain that feeds the strip
            nc.scalar.dma_start(out=taps[:], in_=tp[:, :])

            # one Toeplitz strip tile, filled by column-range DMAs spread
            # over several engines (the cost model charges a DMA to its
            # issuing engine, so these transfer concurrently); the first
            # range is small so PE's first dependency lands early
            strip = consts.tile([KTAP, STRIP_COLS], BF16, tag="strip",
                                name="strip")
            bounds = [0, 2048, 15360, STRIP_COLS]
            strip_eng = [nc.sync, nc.scalar, nc.sync]
            for eng, a, bnd in zip(strip_eng, bounds[:-1], bounds[1:]):
                src = bass.AP(xp, a, [[1, KTAP], [1, bnd - a]])
                eng.dma_start(out=strip[:, a:bnd], in_=src)

            for dg in range(0, MB, DMA_BLOCKS):
                mg = min(DMA_BLOCKS, MB - dg)
                staged = stage_pool.tile([128, mg, C], BF16, tag="staged",
                                         name="staged")
                for bq in range(0, mg, BANK_BLOCKS):
                    nb = min(BANK_BLOCKS, mg - bq)
                    pt = psum_pool.tile([128, nb, C], F32, tag="bank", name="pt")
                    for ms in range(nb):
                        m = dg + bq + ms
                        for (c0, c1, ng) in groups:
                            for b in range(ng):
                                u0 = 128 * m - KTAP * b + OFF0
                                nc.tensor.matmul(
                                    pt[:, ms, c0:c1],
                                    lhsT=strip[:, u0:u0 + 128],
                                    rhs=taps[:, 128 * b + c0:128 * b + c1],
                                    start=(b == 0),
                                    stop=(b == ng - 1),
                                )
                    nc.any.tensor_copy(staged[:, bq:bq + nb, :], pt[:, :, :])
                # output DMAs ride the otherwise-idle GPSIMD queue
                nc.gpsimd.dma_start(out=out[:, dg:dg + mg, :],
                                    in_=staged[:, :, :])
    nc.finalize()
    return nc


def make_tables(coef_re, coef_im, factor):
    h, env = _fir_design(coef_re, coef_im, factor)
    groups, nblk = _plan_groups(env)
    nper = np.empty(C, int)
    for c0, c1, ng in groups:
        nper[c0:c1] = ng
    hz = h.copy()
    for c in range(C):
        hz[c, nper[c] * KTAP:] = 0.0
    # tapsT[p, 128*b + c] = hz[c, KTAP*b + (KTAP-1) - p]
    hb = hz.reshape(C, NMAX, KTAP)         # [c, b, j0]
    tapsT = hb[:, :, ::-1].transpose(2, 1, 0).reshape(KTAP, NMAX * C)
    return np.ascontiguousarray(tapsT.astype(NPBF16)), groups


_CACHE = {}


def kernel(inp, coef_re, coef_im, factor):
    inp = np.ascontiguousarray(np.asarray(inp, np.float32))
    assert inp.shape == (B, T)
    tapsT, groups = make_tables(coef_re, coef_im, factor)

    key = tuple(groups)
    if key not in _CACHE:
        _CACHE[key] = build_bass(groups)
    nc = _CACHE[key]

    xpad = np.zeros((B, XPAD_LEN), np.float32)
    xpad[:, XPAD_OFF:XPAD_OFF + T] = inp
    xpad = xpad.astype(NPBF16)

    in_maps = [
        {"xp": xpad[i:i + 1], "taps": tapsT}
        for i in range(B)
    ]
    res = run_bass_kernel_spmd(nc, in_maps, core_ids=list(range(B)))
    out = np.stack([
        np.asarray(res.results[i]["out"]).astype(np.float32)
        .transpose(1, 0, 2).reshape(T, C)
        for i in range(B)
    ])
    return np.ascontiguousarray(out)


# revision 23
# speedup vs baseline: 6.9620x; 1.0612x over previous
"""Gammatone filterbank on TRN2 as a truncated-FIR matmul (PE engine).

The module is 4 cascaded identical complex one-pole IIR sections per band;
its exact impulse response is h_c[j] = factor_c * C(j+3,3) * lam_c^j *
cos(beta_c * j) (real part; the input is real).  |coef| <= 0.985 so h decays
geometrically: truncating at J_c taps (J_c chosen per band from the tail L2
norm, <= 768) keeps the max error ~1e-3 of output scale -- far inside the
2e-2 gate -- and turns the whole cascade into one batched FIR.

The FIR is evaluated on the Tensor engine: for each 128-sample output block
m, out[t, c] = sum_b lhsT_b^T @ taps_b where lhsT_b[p, t] = x[128(m-b) + t +
p - 127] is a 128x128 slice of a precomputed Toeplitz "strip" S[p, u] =
x_pad[u + p] (one overlapping-AP DMA builds it; bf16), and taps_b[p, c] =
h_c[128 b + 127 - p] (constant, bf16).  Bands need 1..6 tap blocks; bands
are grouped by block count so PE work is ~285 psum rows per output block
(~30 us total) instead of 6*128.  PSUM accumulates in f32; each 2 KiB bank
holds 4 output blocks, is copied (f32->bf16) to an SBUF stage by whichever
engine is free, and staged groups of 16 blocks go to DRAM with one DMA
(1 KiB+ descriptors keep the DMA bus at full model bandwidth).

Output DRAM layout is [t_local, m, c] bf16; the host transposes to
[T, C] f32 (host work is not on the device critical path).  Total DMA is
~8.5 MB strip in + ~8 MB out vs 16 MB out alone for the f32 IIR baseline,
and the serial DVE scan chain (8 scans x 32000 cols at 0.96 GHz ~ 270 us)
disappears entirely.

Sharding: batch-parallel SPMD, one waveform per NeuronCore (8 cores, B=8).
"""

import sys

import numpy as np

for _p in ("/opt/trn_rl_repo",):
    if _p not in sys.path:
        sys.path.insert(0, _p)

import ml_dtypes

import concourse.bass as bass  # noqa: F401
import concourse.mybir as mybir
from concourse.bacc import Bacc
from concourse.bass_utils import run_bass_kernel_spmd
from concourse.tile import TileContext

B = 8
T = 32000
C = 128
MB = T // 128            # 250 output blocks of 128 samples
KTAP = 128               # taps per matmul contraction block (<= 128)
NMAX = 6                 # max tap blocks per band
TOL = 3e-3               # tail L2 threshold for per-band tap count
MIN_GROUP = 8            # merge band groups smaller than this
BANK_BLOCKS = 8          # output blocks per PSUM accumulation tile (2 banks)
DMA_BLOCKS = 32          # output blocks staged per output DMA
STAGE_BUFS = 6
OUT_ENGINES = ("pool", "pool", "pool", "pool", "pool", "sp", "pool", "sp")
BF16 = mybir.dt.bfloat16
F32 = mybir.dt.float32
NPBF16 = ml_dtypes.bfloat16

OFF0 = (NMAX - 1) * KTAP            # strip column of (m=0, b=NMAX-1) window
STRIP_COLS = 128 * (MB - 1) + 128 + OFF0
XPAD_OFF = OFF0 + KTAP - 1          # leading zeros in x_pad
XPAD_LEN = STRIP_COLS + KTAP        # >= STRIP_COLS-1 + (KTAP-1) + 1


def _fir_design(coef_re, coef_im, factor):
    """Exact cascade impulse response h[c, j] and its envelope, j < NMAX*KTAP."""
    cr = np.asarray(coef_re, np.float64)
    ci = np.asarray(coef_im, np.float64)
    f = np.asarray(factor, np.float64)
    lam = np.hypot(cr, ci)
    beta = np.arctan2(ci, cr)
    j = np.arange(NMAX * KTAP, dtype=np.float64)
    cj = (j + 1.0) * (j + 2.0) * (j + 3.0) / 6.0
    env = f[:, None] * cj[None, :] * lam[:, None] ** j[None, :]
    h = env * np.cos(beta[:, None] * j[None, :])
    return h, env


def _plan_groups(env):
    """Per-band tap-block counts -> channel groups [(c0, c1, nblocks)]."""
    tail = np.sqrt((env ** 2)[:, ::-1].cumsum(axis=1))[:, ::-1]
    jreq = (tail > TOL).sum(axis=1)
    nblk = np.clip(np.ceil(jreq / float(KTAP)).astype(int), 1, NMAX)
    # prefix grouping needs nblk non-increasing in c (true for this bank,
    # enforce anyway)
    nblk = np.maximum.accumulate(nblk[::-1])[::-1]
    groups = []
    c0 = 0
    for c in range(1, C + 1):
        if c == C or nblk[c] != nblk[c0]:
            groups.append([c0, c, int(nblk[c0])])
            c0 = c
    # absorb runt groups into a neighbor, keeping the larger block count
    merged = []
    for g in groups:
        if merged and (g[1] - g[0] < MIN_GROUP or merged[-1][1] - merged[-1][0] < MIN_GROUP):
            merged[-1][1] = g[1]
        else:
            merged.append(g)
    return [tuple(g) for g in merged], nblk


def build_bass(groups):
    nc = Bacc()
    xp = nc.declare_dram_parameter("xp", [1, XPAD_LEN], BF16, isOutput=False)
    tp = nc.declare_dram_parameter("taps", [KTAP, NMAX * 128], BF16,
                                   isOutput=False)
    out = nc.declare_dram_parameter("out", [128, MB, C], BF16, isOutput=True)

    with TileContext(nc) as tc:
        with (
            tc.tile_pool(name="consts", bufs=1) as consts,
            tc.tile_pool(name="psum", bufs=16 // BANK_BLOCKS,
                         space="PSUM") as psum_pool,
            tc.tile_pool(name="stage", bufs=STAGE_BUFS) as stage_pool,
        ):
            taps = consts.tile([KTAP, NMAX * 128], BF16, tag="taps",
                               name="taps")
            # taps DMA issued from the Activation engine so its HWDGE/SEQ
            # work stays off the SP chain that feeds the strip
            nc.scalar.dma_start(out=taps[:], in_=tp[:, :])

            # one Toeplitz strip tile, filled by column-range DMAs spread
            # over several engines (the cost model charges a DMA to its
            # issuing engine, so these transfer concurrently); the first
            # range is small so PE's first dependency lands early
            strip = consts.tile([KTAP, STRIP_COLS], BF16, tag="strip",
                                name="strip")
            bounds = [0, 2048]
            while bounds[-1] < STRIP_COLS:
                bounds.append(min(bounds[-1] + 3072, STRIP_COLS))
            for i, (a, bnd) in enumerate(zip(bounds[:-1], bounds[1:])):
                src = bass.AP(xp, a, [[1, KTAP], [1, bnd - a]])
                eng = nc.sync if i % 2 == 0 else nc.scalar
                eng.dma_start(out=strip[:, a:bnd], in_=src)

            for dg in range(0, MB, DMA_BLOCKS):
                mg = min(DMA_BLOCKS, MB - dg)
                staged = stage_pool.tile([128, mg, C], BF16, tag="staged",
                                         name="staged")
                for bq in range(0, mg, BANK_BLOCKS):
                    nb = min(BANK_BLOCKS, mg - bq)
                    pt = psum_pool.tile([128, nb, C], F32, tag="bank", name="pt")
                    for ms in range(nb):
                        m = dg + bq + ms
                        for (c0, c1, ng) in groups:
                            for b in range(ng):
                                u0 = 128 * m - KTAP * b + OFF0
                                nc.tensor.matmul(
                                    pt[:, ms, c0:c1],
                                    lhsT=strip[:, u0:u0 + 128],
                                    rhs=taps[:, 128 * b + c0:128 * b + c1],
                                    start=(b == 0),
                                    stop=(b == ng - 1),
                                )
                    nc.any.tensor_copy(staged[:, bq:bq + nb, :], pt[:, :, :])
                # output DMAs ride the GPSIMD queue, with a couple on SP
                # after its strip chunks have drained
                eng = nc.gpsimd if OUT_ENGINES[(dg // DMA_BLOCKS)
                                               % len(OUT_ENGINES)] == "pool" \
                    else nc.sync
                eng.dma_start(out=out[:, dg:dg + mg, :], in_=staged[:, :, :])
    nc.finalize()
    return nc


def make_tables(coef_re, coef_im, factor):
    h, env = _fir_design(coef_re, coef_im, factor)
    groups, nblk = _plan_groups(env)
    nper = np.empty(C, int)
    for c0, c1, ng in groups:
        nper[c0:c1] = ng
    hz = h.copy()
    for c in range(C):
        hz[c, nper[c] * KTAP:] = 0.0
    # tapsT[p, 128*b + c] = hz[c, KTAP*b + (KTAP-1) - p]
    hb = hz.reshape(C, NMAX, KTAP)         # [c, b, j0]
    tapsT = hb[:, :, ::-1].transpose(2, 1, 0).reshape(KTAP, NMAX * C)
    return np.ascontiguousarray(tapsT.astype(NPBF16)), groups


_CACHE = {}


def kernel(inp, coef_re, coef_im, factor):
    inp = np.ascontiguousarray(np.asarray(inp, np.float32))
    assert inp.shape == (B, T)
    tapsT, groups = make_tables(coef_re, coef_im, factor)

    key = tuple(groups)
    if key not in _CACHE:
        _CACHE[key] = build_bass(groups)
    nc = _CACHE[key]

    xpad = np.zeros((B, XPAD_LEN), np.float32)
    xpad[:, XPAD_OFF:XPAD_OFF + T] = inp
    xpad = xpad.astype(NPBF16)

    in_maps = [
        {"xp": xpad[i:i + 1], "taps": tapsT}
        for i in range(B)
    ]
    res = run_bass_kernel_spmd(nc, in_maps, core_ids=list(range(B)))
    out = np.stack([
        np.asarray(res.results[i]["out"]).astype(np.float32)
        .transpose(1, 0, 2).reshape(T, C)
        for i in range(B)
    ])
    return np.ascontiguousarray(out)


# revision 28
# speedup vs baseline: 8.5346x; 1.2259x over previous
"""Gammatone filterbank on TRN2 as a truncated-FIR matmul (PE engine).

The module is 4 cascaded identical complex one-pole IIR sections per band;
its exact impulse response is h_c[j] = factor_c * C(j+3,3) * lam_c^j *
cos(beta_c * j) (real part; the input is real).  |coef| <= 0.985 so h decays
geometrically: truncating at J_c taps (J_c chosen per band from the tail L2
norm, <= 768) keeps the max error ~1e-3 of output scale -- far inside the
2e-2 gate -- and turns the whole cascade into one batched FIR.

The FIR is evaluated on the Tensor engine: for each 128-sample output block
m, out[t, c] = sum_b lhsT_b^T @ taps_b where lhsT_b[p, t] = x[128(m-b) + t +
p - 127] is a 128x128 slice of a precomputed Toeplitz "strip" S[p, u] =
x_pad[u + p] (one overlapping-AP DMA builds it; bf16), and taps_b[p, c] =
h_c[128 b + 127 - p] (constant, bf16).  Bands need 1..6 tap blocks; bands
are grouped by block count so PE work is ~285 psum rows per output block
(~30 us total) instead of 6*128.  PSUM accumulates in f32; each 2 KiB bank
holds 4 output blocks, is copied (f32->bf16) to an SBUF stage by whichever
engine is free, and staged groups of 16 blocks go to DRAM with one DMA
(1 KiB+ descriptors keep the DMA bus at full model bandwidth).

Output DRAM layout is [t_local, m, c] bf16; the host transposes to
[T, C] f32 (host work is not on the device critical path).  Total DMA is
~8.5 MB strip in + ~8 MB out vs 16 MB out alone for the f32 IIR baseline,
and the serial DVE scan chain (8 scans x 32000 cols at 0.96 GHz ~ 270 us)
disappears entirely.

Sharding: batch-parallel SPMD, one waveform per NeuronCore (8 cores, B=8).
"""

import sys

import numpy as np

for _p in ("/opt/trn_rl_repo",):
    if _p not in sys.path:
        sys.path.insert(0, _p)

import ml_dtypes

import concourse.bass as bass  # noqa: F401
import concourse.mybir as mybir
from concourse.bacc import Bacc
from concourse.bass_utils import run_bass_kernel_spmd
from concourse.tile import TileContext

B = 8
T = 32000
C = 128
MB = T // 128            # 250 output blocks of 128 samples
KTAP = 128               # taps per matmul contraction block (<= 128)
NMAX = 6                 # max tap blocks per band
TOL = 3e-3               # tail L2 threshold for per-band tap count
MIN_GROUP = 8            # merge band groups smaller than this
BANK_BLOCKS = 8          # output blocks per PSUM accumulation tile (2 banks)
DMA_BLOCKS = 32          # output blocks staged per output DMA
STAGE_BUFS = 6
OUT_ENGINES = ("pool", "pool", "pool", "pool", "pool", "pool", "sp", "pool",
               "pool", "pool")
BF16 = mybir.dt.bfloat16
F32 = mybir.dt.float32
NPBF16 = ml_dtypes.bfloat16

OFF0 = (NMAX - 1) * KTAP            # strip column of (m=0, b=NMAX-1) window
STRIP_COLS = 128 * (MB - 1) + 128 + OFF0
XPAD_OFF = OFF0 + KTAP - 1          # leading zeros in x_pad
XPAD_LEN = STRIP_COLS + KTAP        # >= STRIP_COLS-1 + (KTAP-1) + 1


def _fir_design(coef_re, coef_im, factor):
    """Exact cascade impulse response h[c, j] and its envelope, j < NMAX*KTAP."""
    cr = np.asarray(coef_re, np.float64)
    ci = np.asarray(coef_im, np.float64)
    f = np.asarray(factor, np.float64)
    lam = np.hypot(cr, ci)
    beta = np.arctan2(ci, cr)
    j = np.arange(NMAX * KTAP, dtype=np.float64)
    cj = (j + 1.0) * (j + 2.0) * (j + 3.0) / 6.0
    env = f[:, None] * cj[None, :] * lam[:, None] ** j[None, :]
    h = env * np.cos(beta[:, None] * j[None, :])
    return h, env


def _plan_groups(env):
    """Per-band tap-block counts -> channel groups [(c0, c1, nblocks)]."""
    tail = np.sqrt((env ** 2)[:, ::-1].cumsum(axis=1))[:, ::-1]
    jreq = (tail > TOL).sum(axis=1)
    nblk = np.clip(np.ceil(jreq / float(KTAP)).astype(int), 1, NMAX)
    # prefix grouping needs nblk non-increasing in c (true for this bank,
    # enforce anyway)
    nblk = np.maximum.accumulate(nblk[::-1])[::-1]
    groups = []
    c0 = 0
    for c in range(1, C + 1):
        if c == C or nblk[c] != nblk[c0]:
            groups.append([c0, c, int(nblk[c0])])
            c0 = c
    # absorb runt groups into a neighbor, keeping the larger block count
    merged = []
    for g in groups:
        if merged and (g[1] - g[0] < MIN_GROUP or merged[-1][1] - merged[-1][0] < MIN_GROUP):
            merged[-1][1] = g[1]
        else:
            merged.append(g)
    return [tuple(g) for g in merged], nblk


def build_bass(groups):
    nc = Bacc()
    xp = nc.declare_dram_parameter("xp", [1, XPAD_LEN], BF16, isOutput=False)
    tp = nc.declare_dram_parameter("taps", [KTAP, NMAX * 128], BF16,
                                   isOutput=False)
    out = nc.declare_dram_parameter("out", [128, MB, C], BF16, isOutput=True)

    with TileContext(nc) as tc:
        with (
            tc.tile_pool(name="consts", bufs=1) as consts,
            tc.tile_pool(name="psum", bufs=32 // BANK_BLOCKS,
                         space="PSUM") as psum_pool,
            tc.tile_pool(name="stage", bufs=STAGE_BUFS) as stage_pool,
        ):
            taps = consts.tile([KTAP, NMAX * 128], BF16, tag="taps",
                               name="taps")
            # taps DMA issued from the Activation engine so its HWDGE/SEQ
            # work stays off the SP chain that feeds the strip
            nc.scalar.dma_start(out=taps[:], in_=tp[:, :])

            # one Toeplitz strip tile, filled by column-range DMAs spread
            # over several engines (the cost model charges a DMA to its
            # issuing engine, so these transfer concurrently); the first
            # range is small so PE's first dependency lands early
            strip = consts.tile([KTAP, STRIP_COLS], BF16, tag="strip",
                                name="strip")
            bounds = [0, 2048]
            while bounds[-1] < STRIP_COLS:
                bounds.append(min(bounds[-1] + 3072, STRIP_COLS))
            for i, (a, bnd) in enumerate(zip(bounds[:-1], bounds[1:])):
                src = bass.AP(xp, a, [[1, KTAP], [1, bnd - a]])
                eng = nc.scalar if i % 3 == 1 else nc.sync
                eng.dma_start(out=strip[:, a:bnd], in_=src)

            # bulk output groups of DMA_BLOCKS; the tail split finer so the
            # final transfer (and the drain behind it) is short
            sizes = []
            left = MB
            while left > DMA_BLOCKS:
                sizes.append(DMA_BLOCKS)
                left -= DMA_BLOCKS
            while left > 0:
                s = max(BANK_BLOCKS, left - left // 2) if left > BANK_BLOCKS \
                    else left
                sizes.append(s)
                left -= s
            dg = 0
            for gi, mg in enumerate(sizes):
                staged = stage_pool.tile([128, mg, C], BF16, tag="staged",
                                         name="staged")
                for bq in range(0, mg, BANK_BLOCKS):
                    nb = min(BANK_BLOCKS, mg - bq)
                    pt = psum_pool.tile([128, nb, C], F32, tag="bank", name="pt")
                    for ms in range(nb):
                        m = dg + bq + ms
                        for (c0, c1, ng) in groups:
                            for b in range(ng):
                                u0 = 128 * m - KTAP * b + OFF0
                                nc.tensor.matmul(
                                    pt[:, ms, c0:c1],
                                    lhsT=strip[:, u0:u0 + 128],
                                    rhs=taps[:, 128 * b + c0:128 * b + c1],
                                    start=(b == 0),
                                    stop=(b == ng - 1),
                                )
                    nc.any.tensor_copy(staged[:, bq:bq + nb, :], pt[:, :, :])
                # output DMAs ride the GPSIMD queue, with one on SP after
                # its strip chunks have drained
                eng = nc.gpsimd if OUT_ENGINES[gi % len(OUT_ENGINES)] == \
                    "pool" else nc.sync
                eng.dma_start(out=out[:, dg:dg + mg, :], in_=staged[:, :, :])
                dg += mg
    nc.finalize()
    return nc


def make_tables(coef_re, coef_im, factor):
    h, env = _fir_design(coef_re, coef_im, factor)
    groups, nblk = _plan_groups(env)
    nper = np.empty(C, int)
    for c0, c1, ng in groups:
        nper[c0:c1] = ng
    hz = h.copy()
    for c in range(C):
        hz[c, nper[c] * KTAP:] = 0.0
    # tapsT[p, 128*b + c] = hz[c, KTAP*b + (KTAP-1) - p]
    hb = hz.reshape(C, NMAX, KTAP)         # [c, b, j0]
    tapsT = hb[:, :, ::-1].transpose(2, 1, 0).reshape(KTAP, NMAX * C)
    return np.ascontiguousarray(tapsT.astype(NPBF16)), groups


_CACHE = {}


def kernel(inp, coef_re, coef_im, factor):
    inp = np.ascontiguousarray(np.asarray(inp, np.float32))
    assert inp.shape == (B, T)
    tapsT, groups = make_tables(coef_re, coef_im, factor)

    key = tuple(groups)
    if key not in _CACHE:
        _CACHE[key] = build_bass(groups)
    nc = _CACHE[key]

    xpad = np.zeros((B, XPAD_LEN), np.float32)
    xpad[:, XPAD_OFF:XPAD_OFF + T] = inp
    xpad = xpad.astype(NPBF16)

    in_maps = [
        {"xp": xpad[i:i + 1], "taps": tapsT}
        for i in range(B)
    ]
    res = run_bass_kernel_spmd(nc, in_maps, core_ids=list(range(B)))
    out = np.stack([
        np.asarray(res.results[i]["out"]).astype(np.float32)
        .transpose(1, 0, 2).reshape(T, C)
        for i in range(B)
    ])
    return np.ascontiguousarray(out)


# revision 30
# speedup vs baseline: 8.5562x; 1.0025x over previous
"""Gammatone filterbank on TRN2 as a truncated-FIR matmul (PE engine).

The module is 4 cascaded identical complex one-pole IIR sections per band;
its exact impulse response is h_c[j] = factor_c * C(j+3,3) * lam_c^j *
cos(beta_c * j) (real part; the input is real).  |coef| <= 0.985 so h decays
geometrically: truncating at J_c taps (J_c chosen per band from the tail L2
norm, <= 768) keeps the max error ~1e-3 of output scale -- far inside the
2e-2 gate -- and turns the whole cascade into one batched FIR.

The FIR is evaluated on the Tensor engine: for each 128-sample output block
m, out[t, c] = sum_b lhsT_b^T @ taps_b where lhsT_b[p, t] = x[128(m-b) + t +
p - 127] is a 128x128 slice of a precomputed Toeplitz "strip" S[p, u] =
x_pad[u + p] (one overlapping-AP DMA builds it; bf16), and taps_b[p, c] =
h_c[128 b + 127 - p] (constant, bf16).  Bands need 1..6 tap blocks; bands
are grouped by block count so PE work is ~285 psum rows per output block
(~30 us total) instead of 6*128.  PSUM accumulates in f32; each 2 KiB bank
holds 4 output blocks, is copied (f32->bf16) to an SBUF stage by whichever
engine is free, and staged groups of 16 blocks go to DRAM with one DMA
(1 KiB+ descriptors keep the DMA bus at full model bandwidth).

Output DRAM layout is [t_local, m, c] bf16; the host transposes to
[T, C] f32 (host work is not on the device critical path).  Total DMA is
~8.5 MB strip in + ~8 MB out vs 16 MB out alone for the f32 IIR baseline,
and the serial DVE scan chain (8 scans x 32000 cols at 0.96 GHz ~ 270 us)
disappears entirely.

Sharding: batch-parallel SPMD, one waveform per NeuronCore (8 cores, B=8).
"""

import sys

import numpy as np

for _p in ("/opt/trn_rl_repo",):
    if _p not in sys.path:
        sys.path.insert(0, _p)

import ml_dtypes

import concourse.bass as bass  # noqa: F401
import concourse.mybir as mybir
from concourse.bacc import Bacc
from concourse.bass_utils import run_bass_kernel_spmd
from concourse.tile import TileContext

B = 8
T = 32000
C = 128
MB = T // 128            # 250 output blocks of 128 samples
KTAP = 128               # taps per matmul contraction block (<= 128)
NMAX = 6                 # max tap blocks per band
TOL = 3e-3               # tail L2 threshold for per-band tap count
MIN_GROUP = 8            # merge band groups smaller than this
BANK_BLOCKS = 8          # output blocks per PSUM accumulation tile (2 banks)
DMA_BLOCKS = 32          # output blocks staged per output DMA
STAGE_BUFS = 6
OUT_ENGINES = ("pool", "pool", "pool", "pool", "pool", "pool", "sp", "pool",
               "pool", "pool")
BF16 = mybir.dt.bfloat16
F32 = mybir.dt.float32
NPBF16 = ml_dtypes.bfloat16

OFF0 = (NMAX - 1) * KTAP            # strip column of (m=0, b=NMAX-1) window
STRIP_COLS = 128 * (MB - 1) + 128 + OFF0
XPAD_OFF = OFF0 + KTAP - 1          # leading zeros in x_pad
XPAD_LEN = STRIP_COLS + KTAP        # >= STRIP_COLS-1 + (KTAP-1) + 1


def _fir_design(coef_re, coef_im, factor):
    """Exact cascade impulse response h[c, j] and its envelope, j < NMAX*KTAP."""
    cr = np.asarray(coef_re, np.float64)
    ci = np.asarray(coef_im, np.float64)
    f = np.asarray(factor, np.float64)
    lam = np.hypot(cr, ci)
    beta = np.arctan2(ci, cr)
    j = np.arange(NMAX * KTAP, dtype=np.float64)
    cj = (j + 1.0) * (j + 2.0) * (j + 3.0) / 6.0
    env = f[:, None] * cj[None, :] * lam[:, None] ** j[None, :]
    h = env * np.cos(beta[:, None] * j[None, :])
    return h, env


def _plan_groups(env):
    """Per-band tap-block counts -> channel groups [(c0, c1, nblocks)]."""
    tail = np.sqrt((env ** 2)[:, ::-1].cumsum(axis=1))[:, ::-1]
    jreq = (tail > TOL).sum(axis=1)
    nblk = np.clip(np.ceil(jreq / float(KTAP)).astype(int), 1, NMAX)
    # prefix grouping needs nblk non-increasing in c (true for this bank,
    # enforce anyway)
    nblk = np.maximum.accumulate(nblk[::-1])[::-1]
    groups = []
    c0 = 0
    for c in range(1, C + 1):
        if c == C or nblk[c] != nblk[c0]:
            groups.append([c0, c, int(nblk[c0])])
            c0 = c
    # absorb runt groups into a neighbor, keeping the larger block count
    merged = []
    for g in groups:
        if merged and (g[1] - g[0] < MIN_GROUP or merged[-1][1] - merged[-1][0] < MIN_GROUP):
            merged[-1][1] = g[1]
        else:
            merged.append(g)
    return [tuple(g) for g in merged], nblk


def build_bass(groups):
    nc = Bacc()
    xp = nc.declare_dram_parameter("xp", [1, XPAD_LEN], BF16, isOutput=False)
    tp = nc.declare_dram_parameter("taps", [KTAP, NMAX * 128], BF16,
                                   isOutput=False)
    out = nc.declare_dram_parameter("out", [128, MB, C], BF16, isOutput=True)

    with TileContext(nc) as tc:
        with (
            tc.tile_pool(name="consts", bufs=1) as consts,
            tc.tile_pool(name="psum", bufs=32 // BANK_BLOCKS,
                         space="PSUM") as psum_pool,
            tc.tile_pool(name="stage", bufs=STAGE_BUFS) as stage_pool,
        ):
            taps = consts.tile([KTAP, NMAX * 128], BF16, tag="taps",
                               name="taps")
            # taps DMA issued from the Activation engine so its HWDGE/SEQ
            # work stays off the SP chain that feeds the strip
            nc.scalar.dma_start(out=taps[:], in_=tp[:, :])

            # one Toeplitz strip tile, filled by column-range DMAs spread
            # over several engines (the cost model charges a DMA to its
            # issuing engine, so these transfer concurrently); the first
            # range is small so PE's first dependency lands early
            strip = consts.tile([KTAP, STRIP_COLS], BF16, tag="strip",
                                name="strip")
            bounds = [0, 768, 2048]
            while bounds[-1] < STRIP_COLS:
                bounds.append(min(bounds[-1] + 3072, STRIP_COLS))
            for i, (a, bnd) in enumerate(zip(bounds[:-1], bounds[1:])):
                src = bass.AP(xp, a, [[1, KTAP], [1, bnd - a]])
                eng = nc.scalar if i % 3 == 1 else nc.sync
                eng.dma_start(out=strip[:, a:bnd], in_=src)

            # bulk output groups of DMA_BLOCKS; the tail split finer so the
            # final transfer (and the drain behind it) is short
            tail_sizes = [12, 8, 4, 2]
            sizes = []
            left = MB - sum(tail_sizes)
            while left > 0:
                sizes.append(min(DMA_BLOCKS, left))
                left -= sizes[-1]
            sizes += tail_sizes
            dg = 0
            for gi, mg in enumerate(sizes):
                staged = stage_pool.tile([128, mg, C], BF16, tag="staged",
                                         name="staged")
                for bq in range(0, mg, BANK_BLOCKS):
                    nb = min(BANK_BLOCKS, mg - bq)
                    pt = psum_pool.tile([128, nb, C], F32, tag="bank", name="pt")
                    for ms in range(nb):
                        m = dg + bq + ms
                        for (c0, c1, ng) in groups:
                            for b in range(ng):
                                u0 = 128 * m - KTAP * b + OFF0
                                nc.tensor.matmul(
                                    pt[:, ms, c0:c1],
                                    lhsT=strip[:, u0:u0 + 128],
                                    rhs=taps[:, 128 * b + c0:128 * b + c1],
                                    start=(b == 0),
                                    stop=(b == ng - 1),
                                )
                    nc.any.tensor_copy(staged[:, bq:bq + nb, :], pt[:, :, :])
                # output DMAs ride the GPSIMD queue, with one on SP after
                # its strip chunks have drained
                eng = nc.gpsimd if OUT_ENGINES[gi % len(OUT_ENGINES)] == \
                    "pool" else nc.sync
                eng.dma_start(out=out[:, dg:dg + mg, :], in_=staged[:, :, :])
                dg += mg
    nc.finalize()
    return nc


def make_tables(coef_re, coef_im, factor):
    h, env = _fir_design(coef_re, coef_im, factor)
    groups, nblk = _plan_groups(env)
    nper = np.empty(C, int)
    for c0, c1, ng in groups:
        nper[c0:c1] = ng
    hz = h.copy()
    for c in range(C):
        hz[c, nper[c] * KTAP:] = 0.0
    # tapsT[p, 128*b + c] = hz[c, KTAP*b + (KTAP-1) - p]
    hb = hz.reshape(C, NMAX, KTAP)         # [c, b, j0]
    tapsT = hb[:, :, ::-1].transpose(2, 1, 0).reshape(KTAP, NMAX * C)
    return np.ascontiguousarray(tapsT.astype(NPBF16)), groups


_CACHE = {}


def kernel(inp, coef_re, coef_im, factor):
    inp = np.ascontiguousarray(np.asarray(inp, np.float32))
    assert inp.shape == (B, T)
    tapsT, groups = make_tables(coef_re, coef_im, factor)

    key = tuple(groups)
    if key not in _CACHE:
        _CACHE[key] = build_bass(groups)
    nc = _CACHE[key]

    xpad = np.zeros((B, XPAD_LEN), np.float32)
    xpad[:, XPAD_OFF:XPAD_OFF + T] = inp
    xpad = xpad.astype(NPBF16)

    in_maps = [
        {"xp": xpad[i:i + 1], "taps": tapsT}
        for i in range(B)
    ]
    res = run_bass_kernel_spmd(nc, in_maps, core_ids=list(range(B)))
    out = np.stack([
        np.asarray(res.results[i]["out"]).astype(np.float32)
        .transpose(1, 0, 2).reshape(T, C)
        for i in range(B)
    ])
    return np.ascontiguousarray(out)


# revision 33
# speedup vs baseline: 8.9150x; 1.0419x over previous
"""Gammatone filterbank on TRN2 as a truncated-FIR matmul (PE engine).

The module is 4 cascaded identical complex one-pole IIR sections per band;
its exact impulse response is h_c[j] = factor_c * C(j+3,3) * lam_c^j *
cos(beta_c * j) (real part; the input is real).  |coef| <= 0.985 so h decays
geometrically: truncating at J_c taps (J_c chosen per band from the tail L2
norm, <= 768) keeps the max error ~1e-3 of output scale -- far inside the
2e-2 gate -- and turns the whole cascade into one batched FIR.

The FIR is evaluated on the Tensor engine: for each 128-sample output block
m, out[t, c] = sum_b lhsT_b^T @ taps_b where lhsT_b[p, t] = x[128(m-b) + t +
p - 127] is a 128x128 slice of a precomputed Toeplitz "strip" S[p, u] =
x_pad[u + p] (one overlapping-AP DMA builds it; bf16), and taps_b[p, c] =
h_c[128 b + 127 - p] (constant, bf16).  Bands need 1..6 tap blocks; bands
are grouped by block count so PE work is ~285 psum rows per output block
(~30 us total) instead of 6*128.  PSUM accumulates in f32; each 2 KiB bank
holds 4 output blocks, is copied (f32->bf16) to an SBUF stage by whichever
engine is free, and staged groups of 16 blocks go to DRAM with one DMA
(1 KiB+ descriptors keep the DMA bus at full model bandwidth).

Output DRAM layout is [t_local, m, c] bf16; the host transposes to
[T, C] f32 (host work is not on the device critical path).  Total DMA is
~8.5 MB strip in + ~8 MB out vs 16 MB out alone for the f32 IIR baseline,
and the serial DVE scan chain (8 scans x 32000 cols at 0.96 GHz ~ 270 us)
disappears entirely.

Sharding: batch-parallel SPMD, one waveform per NeuronCore (8 cores, B=8).
"""

import sys

import numpy as np

for _p in ("/opt/trn_rl_repo",):
    if _p not in sys.path:
        sys.path.insert(0, _p)

import ml_dtypes

import concourse.bass as bass  # noqa: F401
import concourse.mybir as mybir
from concourse.bacc import Bacc
from concourse.bass_utils import run_bass_kernel_spmd
from concourse.tile import TileContext

B = 8
T = 32000
C = 128
MB = T // 128            # 250 output blocks of 128 samples
KTAP = 128               # taps per matmul contraction block (<= 128)
NMAX = 6                 # max tap blocks per band
TOL = 3e-3               # tail L2 threshold for per-band tap count
MIN_GROUP = 8            # merge band groups smaller than this
BANK_BLOCKS = 8          # output blocks per PSUM accumulation tile (2 banks)
DMA_BLOCKS = 32          # output blocks staged per output DMA
STAGE_BUFS = 6
OUT_ENGINES = ("pool", "pool", "pool", "pool", "pool", "pool", "sp", "pool",
               "pool", "pool")
BF16 = mybir.dt.bfloat16
F32 = mybir.dt.float32
NPBF16 = ml_dtypes.bfloat16

OFF0 = (NMAX - 1) * KTAP            # strip column of (m=0, b=NMAX-1) window
STRIP_COLS = 128 * (MB - 1) + 128 + OFF0
XPAD_OFF = OFF0 + KTAP - 1          # leading zeros in x_pad
XPAD_LEN = STRIP_COLS + KTAP        # >= STRIP_COLS-1 + (KTAP-1) + 1


def _fir_design(coef_re, coef_im, factor):
    """Exact cascade impulse response h[c, j] and its envelope, j < NMAX*KTAP."""
    cr = np.asarray(coef_re, np.float64)
    ci = np.asarray(coef_im, np.float64)
    f = np.asarray(factor, np.float64)
    lam = np.hypot(cr, ci)
    beta = np.arctan2(ci, cr)
    j = np.arange(NMAX * KTAP, dtype=np.float64)
    cj = (j + 1.0) * (j + 2.0) * (j + 3.0) / 6.0
    env = f[:, None] * cj[None, :] * lam[:, None] ** j[None, :]
    h = env * np.cos(beta[:, None] * j[None, :])
    return h, env


def _plan_groups(env):
    """Per-band tap-block counts -> channel groups [(c0, c1, nblocks)]."""
    tail = np.sqrt((env ** 2)[:, ::-1].cumsum(axis=1))[:, ::-1]
    jreq = (tail > TOL).sum(axis=1)
    nblk = np.clip(np.ceil(jreq / float(KTAP)).astype(int), 1, NMAX)
    # prefix grouping needs nblk non-increasing in c (true for this bank,
    # enforce anyway)
    nblk = np.maximum.accumulate(nblk[::-1])[::-1]
    groups = []
    c0 = 0
    for c in range(1, C + 1):
        if c == C or nblk[c] != nblk[c0]:
            groups.append([c0, c, int(nblk[c0])])
            c0 = c
    # absorb runt groups into a neighbor, keeping the larger block count
    merged = []
    for g in groups:
        if merged and (g[1] - g[0] < MIN_GROUP or merged[-1][1] - merged[-1][0] < MIN_GROUP):
            merged[-1][1] = g[1]
        else:
            merged.append(g)
    return [tuple(g) for g in merged], nblk


def build_bass(groups):
    nc = Bacc()
    xp = nc.declare_dram_parameter("xp", [1, XPAD_LEN], BF16, isOutput=False)
    tp = nc.declare_dram_parameter("taps", [KTAP, NMAX * 128], BF16,
                                   isOutput=False)
    out = nc.declare_dram_parameter("out", [128, MB, C], BF16, isOutput=True)

    with TileContext(nc) as tc:
        with (
            tc.tile_pool(name="consts", bufs=1) as consts,
            tc.tile_pool(name="psum", bufs=32 // BANK_BLOCKS,
                         space="PSUM") as psum_pool,
            tc.tile_pool(name="stage", bufs=STAGE_BUFS) as stage_pool,
        ):
            taps = consts.tile([KTAP, NMAX * 128], BF16, tag="taps",
                               name="taps")
            # taps DMA on the (initially idle) GPSIMD queue, off the
            # SP/Act chains that feed the strip
            nc.gpsimd.dma_start(out=taps[:], in_=tp[:, :])

            # one Toeplitz strip tile, filled by column-range DMAs spread
            # over several engines (the cost model charges a DMA to its
            # issuing engine, so these transfer concurrently); the first
            # range is small so PE's first dependency lands early
            strip = consts.tile([KTAP, STRIP_COLS], BF16, tag="strip",
                                name="strip")
            bounds = [0, 768, 2048]
            while bounds[-1] < STRIP_COLS:
                bounds.append(min(bounds[-1] + 3072, STRIP_COLS))
            for i, (a, bnd) in enumerate(zip(bounds[:-1], bounds[1:])):
                src = bass.AP(xp, a, [[1, KTAP], [1, bnd - a]])
                # first two chunks concurrently on SP + Act, then 2:1 SP/Act
                eng = nc.scalar if i % 3 == 1 else nc.sync
                eng.dma_start(out=strip[:, a:bnd], in_=src)

            # bulk output groups of DMA_BLOCKS; the tail split finer so the
            # final transfer (and the drain behind it) is short
            tail_sizes = [12, 8, 4, 2]
            sizes = []
            left = MB - sum(tail_sizes)
            while left > 0:
                sizes.append(min(DMA_BLOCKS, left))
                left -= sizes[-1]
            sizes += tail_sizes
            dg = 0
            for gi, mg in enumerate(sizes):
                staged = stage_pool.tile([128, mg, C], BF16, tag="staged",
                                         name="staged")
                for bq in range(0, mg, BANK_BLOCKS):
                    nb = min(BANK_BLOCKS, mg - bq)
                    pt = psum_pool.tile([128, nb, C], F32, tag="bank", name="pt")
                    for ms in range(nb):
                        m = dg + bq + ms
                        for (c0, c1, ng) in groups:
                            for b in range(ng):
                                u0 = 128 * m - KTAP * b + OFF0
                                nc.tensor.matmul(
                                    pt[:, ms, c0:c1],
                                    lhsT=strip[:, u0:u0 + 128],
                                    rhs=taps[:, 128 * b + c0:128 * b + c1],
                                    start=(b == 0),
                                    stop=(b == ng - 1),
                                )
                    nc.any.tensor_copy(staged[:, bq:bq + nb, :], pt[:, :, :])
                # bulk output DMAs ride the GPSIMD queue (one on SP after
                # its strip chunks drain); the small tail groups fan out
                # over different queues so they flush concurrently
                n_tail = len(tail_sizes)
                if gi >= len(sizes) - n_tail:
                    eng = [nc.gpsimd, nc.sync, nc.scalar,
                           nc.sync][(gi - (len(sizes) - n_tail)) % 4]
                else:
                    eng = nc.gpsimd if OUT_ENGINES[gi % len(OUT_ENGINES)] \
                        == "pool" else nc.sync
                eng.dma_start(out=out[:, dg:dg + mg, :], in_=staged[:, :, :])
                dg += mg
    nc.finalize()
    return nc


def make_tables(coef_re, coef_im, factor):
    h, env = _fir_design(coef_re, coef_im, factor)
    groups, nblk = _plan_groups(env)
    nper = np.empty(C, int)
    for c0, c1, ng in groups:
        nper[c0:c1] = ng
    hz = h.copy()
    for c in range(C):
        hz[c, nper[c] * KTAP:] = 0.0
    # tapsT[p, 128*b + c] = hz[c, KTAP*b + (KTAP-1) - p]
    hb = hz.reshape(C, NMAX, KTAP)         # [c, b, j0]
    tapsT = hb[:, :, ::-1].transpose(2, 1, 0).reshape(KTAP, NMAX * C)
    return np.ascontiguousarray(tapsT.astype(NPBF16)), groups


_CACHE = {}


def kernel(inp, coef_re, coef_im, factor):
    inp = np.ascontiguousarray(np.asarray(inp, np.float32))
    assert inp.shape == (B, T)
    tapsT, groups = make_tables(coef_re, coef_im, factor)

    key = tuple(groups)
    if key not in _CACHE:
        _CACHE[key] = build_bass(groups)
    nc = _CACHE[key]

    xpad = np.zeros((B, XPAD_LEN), np.float32)
    xpad[:, XPAD_OFF:XPAD_OFF + T] = inp
    xpad = xpad.astype(NPBF16)

    in_maps = [
        {"xp": xpad[i:i + 1], "taps": tapsT}
        for i in range(B)
    ]
    res = run_bass_kernel_spmd(nc, in_maps, core_ids=list(range(B)))
    out = np.stack([
        np.asarray(res.results[i]["out"]).astype(np.float32)
        .transpose(1, 0, 2).reshape(T, C)
        for i in range(B)
    ])
    return np.ascontiguousarray(out)


# revision 36
# speedup vs baseline: 9.1148x; 1.0224x over previous
"""Gammatone filterbank on TRN2 as a truncated-FIR matmul (PE engine).

The module is 4 cascaded identical complex one-pole IIR sections per band;
its exact impulse response is h_c[j] = factor_c * C(j+3,3) * lam_c^j *
cos(beta_c * j) (real part; the input is real).  |coef| <= 0.985 so h decays
geometrically: truncating at J_c taps (J_c chosen per band from the tail L2
norm, <= 768) keeps the max error ~1e-3 of output scale -- far inside the
2e-2 gate -- and turns the whole cascade into one batched FIR.

The FIR is evaluated on the Tensor engine: for each 128-sample output block
m, out[t, c] = sum_b lhsT_b^T @ taps_b where lhsT_b[p, t] = x[128(m-b) + t +
p - 127] is a 128x128 slice of a precomputed Toeplitz "strip" S[p, u] =
x_pad[u + p] (one overlapping-AP DMA builds it; bf16), and taps_b[p, c] =
h_c[128 b + 127 - p] (constant, bf16).  Bands need 1..6 tap blocks; bands
are grouped by block count so PE work is ~285 psum rows per output block
(~30 us total) instead of 6*128.  PSUM accumulates in f32; each 2 KiB bank
holds 4 output blocks, is copied (f32->bf16) to an SBUF stage by whichever
engine is free, and staged groups of 16 blocks go to DRAM with one DMA
(1 KiB+ descriptors keep the DMA bus at full model bandwidth).

Output DRAM layout is [t_local, m, c] bf16; the host transposes to
[T, C] f32 (host work is not on the device critical path).  Total DMA is
~8.5 MB strip in + ~8 MB out vs 16 MB out alone for the f32 IIR baseline,
and the serial DVE scan chain (8 scans x 32000 cols at 0.96 GHz ~ 270 us)
disappears entirely.

Sharding: batch-parallel SPMD, one waveform per NeuronCore (8 cores, B=8).
"""

import sys

import numpy as np

for _p in ("/opt/trn_rl_repo",):
    if _p not in sys.path:
        sys.path.insert(0, _p)

import ml_dtypes

import concourse.bass as bass  # noqa: F401
import concourse.mybir as mybir
from concourse.bacc import Bacc
from concourse.bass_utils import run_bass_kernel_spmd
from concourse.tile import TileContext

B = 8
T = 32000
C = 128
MB = T // 128            # 250 output blocks of 128 samples
KTAP = 128               # taps per matmul contraction block (<= 128)
NMAX = 6                 # max tap blocks per band
TOL = 3e-3               # tail L2 threshold for per-band tap count
MIN_GROUP = 4            # merge band groups smaller than this
BANK_BLOCKS = 8          # output blocks per PSUM accumulation tile (2 banks)
DMA_BLOCKS = 32          # output blocks staged per output DMA
STAGE_BUFS = 6
OUT_ENGINES = ("pool", "pool", "pool", "pool", "pool", "pool", "sp", "pool",
               "pool", "pool")
BF16 = mybir.dt.bfloat16
F32 = mybir.dt.float32
NPBF16 = ml_dtypes.bfloat16

OFF0 = (NMAX - 1) * KTAP            # strip column of (m=0, b=NMAX-1) window
STRIP_COLS = 128 * (MB - 1) + 128 + OFF0
XPAD_OFF = OFF0 + KTAP - 1          # leading zeros in x_pad
XPAD_LEN = STRIP_COLS + KTAP        # >= STRIP_COLS-1 + (KTAP-1) + 1


def _fir_design(coef_re, coef_im, factor):
    """Exact cascade impulse response h[c, j] and its envelope, j < NMAX*KTAP."""
    cr = np.asarray(coef_re, np.float64)
    ci = np.asarray(coef_im, np.float64)
    f = np.asarray(factor, np.float64)
    lam = np.hypot(cr, ci)
    beta = np.arctan2(ci, cr)
    j = np.arange(NMAX * KTAP, dtype=np.float64)
    cj = (j + 1.0) * (j + 2.0) * (j + 3.0) / 6.0
    env = f[:, None] * cj[None, :] * lam[:, None] ** j[None, :]
    h = env * np.cos(beta[:, None] * j[None, :])
    return h, env


def _plan_groups(env):
    """Per-band tap-block counts -> channel groups [(c0, c1, nblocks)]."""
    tail = np.sqrt((env ** 2)[:, ::-1].cumsum(axis=1))[:, ::-1]
    jreq = (tail > TOL).sum(axis=1)
    nblk = np.clip(np.ceil(jreq / float(KTAP)).astype(int), 1, NMAX)
    # prefix grouping needs nblk non-increasing in c (true for this bank,
    # enforce anyway)
    nblk = np.maximum.accumulate(nblk[::-1])[::-1]
    groups = []
    c0 = 0
    for c in range(1, C + 1):
        if c == C or nblk[c] != nblk[c0]:
            groups.append([c0, c, int(nblk[c0])])
            c0 = c
    # absorb runt groups into a neighbor, keeping the larger block count
    merged = []
    for g in groups:
        if merged and (g[1] - g[0] < MIN_GROUP or merged[-1][1] - merged[-1][0] < MIN_GROUP):
            merged[-1][1] = g[1]
        else:
            merged.append(g)
    return [tuple(g) for g in merged], nblk


def build_bass(groups):
    nc = Bacc()
    xp = nc.declare_dram_parameter("xp", [1, XPAD_LEN], BF16, isOutput=False)
    tp = nc.declare_dram_parameter("taps", [KTAP, NMAX * 128], BF16,
                                   isOutput=False)
    out = nc.declare_dram_parameter("out", [128, MB, C], BF16, isOutput=True)

    with TileContext(nc) as tc:
        with (
            tc.tile_pool(name="consts", bufs=1) as consts,
            tc.tile_pool(name="psum", bufs=32 // BANK_BLOCKS,
                         space="PSUM") as psum_pool,
            tc.tile_pool(name="stage", bufs=STAGE_BUFS) as stage_pool,
        ):
            taps = consts.tile([KTAP, NMAX * 128], BF16, tag="taps",
                               name="taps")
            # taps DMA on the (initially idle) GPSIMD queue, off the
            # SP/Act chains that feed the strip
            nc.gpsimd.dma_start(out=taps[:], in_=tp[:, :])

            # one Toeplitz strip tile, filled by column-range DMAs spread
            # over several engines (the cost model charges a DMA to its
            # issuing engine, so these transfer concurrently); the first
            # range is small so PE's first dependency lands early
            strip = consts.tile([KTAP, STRIP_COLS], BF16, tag="strip",
                                name="strip")
            bounds = [0, 768, 2048]
            while bounds[-1] < STRIP_COLS:
                bounds.append(min(bounds[-1] + 3072, STRIP_COLS))
            for i, (a, bnd) in enumerate(zip(bounds[:-1], bounds[1:])):
                src = bass.AP(xp, a, [[1, KTAP], [1, bnd - a]])
                # first two chunks concurrently on SP + Act, then 2:1 SP/Act
                eng = nc.scalar if i % 3 == 1 else nc.sync
                eng.dma_start(out=strip[:, a:bnd], in_=src)

            # bulk output groups of DMA_BLOCKS; the tail split finer so the
            # final transfer (and the drain behind it) is short
            # bulk groups of DMA_BLOCKS, then descending sizes so each late
            # transfer is short and flushes right after its data is ready
            tail_sizes = [28, 20, 16, 12, 8, 4, 2]
            sizes = []
            left = MB - sum(tail_sizes)
            while left > 0:
                sizes.append(min(DMA_BLOCKS, left))
                left -= sizes[-1]
            sizes += tail_sizes
            dg = 0
            for gi, mg in enumerate(sizes):
                staged = stage_pool.tile([128, mg, C], BF16, tag="staged",
                                         name="staged")
                for bq in range(0, mg, BANK_BLOCKS):
                    nb = min(BANK_BLOCKS, mg - bq)
                    pt = psum_pool.tile([128, nb, C], F32, tag="bank", name="pt")
                    for ms in range(nb):
                        m = dg + bq + ms
                        for (c0, c1, ng) in groups:
                            for b in range(ng):
                                u0 = 128 * m - KTAP * b + OFF0
                                nc.tensor.matmul(
                                    pt[:, ms, c0:c1],
                                    lhsT=strip[:, u0:u0 + 128],
                                    rhs=taps[:, 128 * b + c0:128 * b + c1],
                                    start=(b == 0),
                                    stop=(b == ng - 1),
                                )
                    nc.any.tensor_copy(staged[:, bq:bq + nb, :], pt[:, :, :])
                # bulk output DMAs ride the GPSIMD queue; late groups
                # alternate GPSIMD/SP so consecutive flushes overlap
                n_tail = len(tail_sizes)
                if gi >= len(sizes) - n_tail:
                    eng = nc.sync if (gi - (len(sizes) - n_tail)) % 2 \
                        else nc.gpsimd
                else:
                    eng = nc.gpsimd
                eng.dma_start(out=out[:, dg:dg + mg, :], in_=staged[:, :, :])
                dg += mg
    nc.finalize()
    return nc


def make_tables(coef_re, coef_im, factor):
    h, env = _fir_design(coef_re, coef_im, factor)
    groups, nblk = _plan_groups(env)
    nper = np.empty(C, int)
    for c0, c1, ng in groups:
        nper[c0:c1] = ng
    hz = h.copy()
    for c in range(C):
        hz[c, nper[c] * KTAP:] = 0.0
    # tapsT[p, 128*b + c] = hz[c, KTAP*b + (KTAP-1) - p]
    hb = hz.reshape(C, NMAX, KTAP)         # [c, b, j0]
    tapsT = hb[:, :, ::-1].transpose(2, 1, 0).reshape(KTAP, NMAX * C)
    return np.ascontiguousarray(tapsT.astype(NPBF16)), groups


_CACHE = {}


def kernel(inp, coef_re, coef_im, factor):
    inp = np.ascontiguousarray(np.asarray(inp, np.float32))
    assert inp.shape == (B, T)
    tapsT, groups = make_tables(coef_re, coef_im, factor)

    key = tuple(groups)
    if key not in _CACHE:
        _CACHE[key] = build_bass(groups)
    nc = _CACHE[key]

    xpad = np.zeros((B, XPAD_LEN), np.float32)
    xpad[:, XPAD_OFF:XPAD_OFF + T] = inp
    xpad = xpad.astype(NPBF16)

    in_maps = [
        {"xp": xpad[i:i + 1], "taps": tapsT}
        for i in range(B)
    ]
    res = run_bass_kernel_spmd(nc, in_maps, core_ids=list(range(B)))
    out = np.stack([
        np.asarray(res.results[i]["out"]).astype(np.float32)
        .transpose(1, 0, 2).reshape(T, C)
        for i in range(B)
    ])
    return np.ascontiguousarray(out)


# revision 41
# speedup vs baseline: 9.3035x; 1.0207x over previous
"""Gammatone filterbank on TRN2 as a truncated-FIR matmul (PE engine).

The module is 4 cascaded identical complex one-pole IIR sections per band;
its exact impulse response is h_c[j] = factor_c * C(j+3,3) * lam_c^j *
cos(beta_c * j) (real part; the input is real).  |coef| <= 0.985 so h decays
geometrically: truncating at J_c taps (J_c chosen per band from the tail L2
norm, <= 768) keeps the max error ~1e-3 of output scale -- far inside the
2e-2 gate -- and turns the whole cascade into one batched FIR.

The FIR is evaluated on the Tensor engine: for each 128-sample output block
m, out[t, c] = sum_b lhsT_b^T @ taps_b where lhsT_b[p, t] = x[128(m-b) + t +
p - 127] is a 128x128 slice of a precomputed Toeplitz "strip" S[p, u] =
x_pad[u + p] (one overlapping-AP DMA builds it; bf16), and taps_b[p, c] =
h_c[128 b + 127 - p] (constant, bf16).  Bands need 1..6 tap blocks; bands
are grouped by block count so PE work is ~285 psum rows per output block
(~30 us total) instead of 6*128.  PSUM accumulates in f32; each 2 KiB bank
holds 4 output blocks, is copied (f32->bf16) to an SBUF stage by whichever
engine is free, and staged groups of 16 blocks go to DRAM with one DMA
(1 KiB+ descriptors keep the DMA bus at full model bandwidth).

Output DRAM layout is [t_local, m, c] bf16; the host transposes to
[T, C] f32 (host work is not on the device critical path).  Total DMA is
~8.5 MB strip in + ~8 MB out vs 16 MB out alone for the f32 IIR baseline,
and the serial DVE scan chain (8 scans x 32000 cols at 0.96 GHz ~ 270 us)
disappears entirely.

Sharding: batch-parallel SPMD, one waveform per NeuronCore (8 cores, B=8).
"""

import sys

import numpy as np

for _p in ("/opt/trn_rl_repo",):
    if _p not in sys.path:
        sys.path.insert(0, _p)

import ml_dtypes

import concourse.bass as bass  # noqa: F401
import concourse.mybir as mybir
from concourse.bacc import Bacc
from concourse.bass_utils import run_bass_kernel_spmd
from concourse.tile import TileContext

B = 8
T = 32000
C = 128
MB = T // 128            # 250 output blocks of 128 samples
KTAP = 128               # taps per matmul contraction block (<= 128)
NMAX = 6                 # max tap blocks per band
TOL = 3e-3               # tail L2 threshold for per-band tap count
MIN_GROUP = 4            # merge band groups smaller than this
BANK_BLOCKS = 8          # output blocks per PSUM accumulation tile (2 banks)
DMA_BLOCKS = 32          # output blocks staged per output DMA
STAGE_BUFS = 6
TAIL_SIZES = (24, 20, 16, 12, 8, 6, 4, 2)
TAIL_ENGS = ("pool", "sp", "pool", "sp", "pool", "sp", "pool", "sp")
STRIP_PAT = (0, 1, 0, 1, 0, 1, 0, 1, 0, 0, 1, 0, 0)  # 0=SP 1=Act per chunk
BF16 = mybir.dt.bfloat16
F32 = mybir.dt.float32
NPBF16 = ml_dtypes.bfloat16

OFF0 = (NMAX - 1) * KTAP            # strip column of (m=0, b=NMAX-1) window
STRIP_COLS = 128 * (MB - 1) + 128 + OFF0
XPAD_OFF = OFF0 + KTAP - 1          # leading zeros in x_pad
XPAD_LEN = STRIP_COLS + KTAP        # >= STRIP_COLS-1 + (KTAP-1) + 1


def _fir_design(coef_re, coef_im, factor):
    """Exact cascade impulse response h[c, j] and its envelope, j < NMAX*KTAP."""
    cr = np.asarray(coef_re, np.float64)
    ci = np.asarray(coef_im, np.float64)
    f = np.asarray(factor, np.float64)
    lam = np.hypot(cr, ci)
    beta = np.arctan2(ci, cr)
    j = np.arange(NMAX * KTAP, dtype=np.float64)
    cj = (j + 1.0) * (j + 2.0) * (j + 3.0) / 6.0
    env = f[:, None] * cj[None, :] * lam[:, None] ** j[None, :]
    h = env * np.cos(beta[:, None] * j[None, :])
    return h, env


def _plan_groups(env):
    """Per-band tap-block counts -> channel groups [(c0, c1, nblocks)]."""
    tail = np.sqrt((env ** 2)[:, ::-1].cumsum(axis=1))[:, ::-1]
    jreq = (tail > TOL).sum(axis=1)
    nblk = np.clip(np.ceil(jreq / float(KTAP)).astype(int), 1, NMAX)
    # prefix grouping needs nblk non-increasing in c (true for this bank,
    # enforce anyway)
    nblk = np.maximum.accumulate(nblk[::-1])[::-1]
    groups = []
    c0 = 0
    for c in range(1, C + 1):
        if c == C or nblk[c] != nblk[c0]:
            groups.append([c0, c, int(nblk[c0])])
            c0 = c
    # absorb runt groups into a neighbor, keeping the larger block count
    merged = []
    for g in groups:
        if merged and (g[1] - g[0] < MIN_GROUP or merged[-1][1] - merged[-1][0] < MIN_GROUP):
            merged[-1][1] = g[1]
        else:
            merged.append(g)
    return [tuple(g) for g in merged], nblk


def build_bass(groups):
    nc = Bacc()
    xp = nc.declare_dram_parameter("xp", [1, XPAD_LEN], BF16, isOutput=False)
    tp = nc.declare_dram_parameter("taps", [KTAP, NMAX * 128], BF16,
                                   isOutput=False)
    out = nc.declare_dram_parameter("out", [128, MB, C], BF16, isOutput=True)

    with TileContext(nc) as tc:
        with (
            tc.tile_pool(name="consts", bufs=1) as consts,
            tc.tile_pool(name="psum", bufs=32 // BANK_BLOCKS,
                         space="PSUM") as psum_pool,
            tc.tile_pool(name="stage", bufs=STAGE_BUFS) as stage_pool,
        ):
            taps = consts.tile([KTAP, NMAX * 128], BF16, tag="taps",
                               name="taps")
            # taps DMA on the (initially idle) GPSIMD queue, off the
            # SP/Act chains that feed the strip
            nc.gpsimd.dma_start(out=taps[:], in_=tp[:, :])

            # one Toeplitz strip tile, filled by column-range DMAs spread
            # over several engines (the cost model charges a DMA to its
            # issuing engine, so these transfer concurrently); the first
            # range is small so PE's first dependency lands early
            strip = consts.tile([KTAP, STRIP_COLS], BF16, tag="strip",
                                name="strip")
            bounds = [0, 768, 2048]
            while bounds[-1] < STRIP_COLS:
                bounds.append(min(bounds[-1] + 3072, STRIP_COLS))
            for i, (a, bnd) in enumerate(zip(bounds[:-1], bounds[1:])):
                src = bass.AP(xp, a, [[1, KTAP], [1, bnd - a]])
                pat = STRIP_PAT[i] if i < len(STRIP_PAT) else (i % 3 == 1)
                eng = nc.scalar if pat else nc.sync
                eng.dma_start(out=strip[:, a:bnd], in_=src)

            # bulk output groups of DMA_BLOCKS; the tail split finer so the
            # final transfer (and the drain behind it) is short
            # bulk groups of DMA_BLOCKS, then descending sizes so each late
            # transfer is short and flushes right after its data is ready
            tail_sizes = list(TAIL_SIZES)
            sizes = []
            left = MB - sum(tail_sizes)
            while left > 0:
                sizes.append(min(DMA_BLOCKS, left))
                left -= sizes[-1]
            sizes += tail_sizes
            dg = 0
            for gi, mg in enumerate(sizes):
                staged = stage_pool.tile([128, mg, C], BF16, tag="staged",
                                         name="staged")
                for bq in range(0, mg, BANK_BLOCKS):
                    nb = min(BANK_BLOCKS, mg - bq)
                    pt = psum_pool.tile([128, nb, C], F32, tag="bank", name="pt")
                    for ms in range(nb):
                        m = dg + bq + ms
                        for (c0, c1, ng) in groups:
                            for b in range(ng):
                                u0 = 128 * m - KTAP * b + OFF0
                                nc.tensor.matmul(
                                    pt[:, ms, c0:c1],
                                    lhsT=strip[:, u0:u0 + 128],
                                    rhs=taps[:, 128 * b + c0:128 * b + c1],
                                    start=(b == 0),
                                    stop=(b == ng - 1),
                                )
                    nc.any.tensor_copy(staged[:, bq:bq + nb, :], pt[:, :, :])
                # bulk output DMAs ride the GPSIMD queue; late groups
                # alternate GPSIMD/SP so consecutive flushes overlap
                n_tail = len(tail_sizes)
                by_name = {"sp": nc.sync, "pool": nc.gpsimd,
                           "act": nc.scalar}
                if gi >= len(sizes) - n_tail:
                    eng = by_name[TAIL_ENGS[gi - (len(sizes) - n_tail)]]
                else:
                    eng = nc.gpsimd
                eng.dma_start(out=out[:, dg:dg + mg, :], in_=staged[:, :, :])
                dg += mg
    nc.finalize()
    return nc


def make_tables(coef_re, coef_im, factor):
    h, env = _fir_design(coef_re, coef_im, factor)
    groups, nblk = _plan_groups(env)
    nper = np.empty(C, int)
    for c0, c1, ng in groups:
        nper[c0:c1] = ng
    hz = h.copy()
    for c in range(C):
        hz[c, nper[c] * KTAP:] = 0.0
    # tapsT[p, 128*b + c] = hz[c, KTAP*b + (KTAP-1) - p]
    hb = hz.reshape(C, NMAX, KTAP)         # [c, b, j0]
    tapsT = hb[:, :, ::-1].transpose(2, 1, 0).reshape(KTAP, NMAX * C)
    return np.ascontiguousarray(tapsT.astype(NPBF16)), groups


_CACHE = {}


def kernel(inp, coef_re, coef_im, factor):
    inp = np.ascontiguousarray(np.asarray(inp, np.float32))
    assert inp.shape == (B, T)
    tapsT, groups = make_tables(coef_re, coef_im, factor)

    key = tuple(groups)
    if key not in _CACHE:
        _CACHE[key] = build_bass(groups)
    nc = _CACHE[key]

    xpad = np.zeros((B, XPAD_LEN), np.float32)
    xpad[:, XPAD_OFF:XPAD_OFF + T] = inp
    xpad = xpad.astype(NPBF16)

    in_maps = [
        {"xp": xpad[i:i + 1], "taps": tapsT}
        for i in range(B)
    ]
    res = run_bass_kernel_spmd(nc, in_maps, core_ids=list(range(B)))
    out = np.stack([
        np.asarray(res.results[i]["out"]).astype(np.float32)
        .transpose(1, 0, 2).reshape(T, C)
        for i in range(B)
    ])
    return np.ascontiguousarray(out)


# revision 43
# speedup vs baseline: 9.7699x; 1.0501x over previous
"""Gammatone filterbank on TRN2 as a truncated-FIR matmul (PE engine).

The module is 4 cascaded identical complex one-pole IIR sections per band;
its exact impulse response is h_c[j] = factor_c * C(j+3,3) * lam_c^j *
cos(beta_c * j) (real part; the input is real).  |coef| <= 0.985 so h decays
geometrically: truncating at J_c taps (J_c chosen per band from the tail L2
norm, <= 768) keeps the max error ~1e-3 of output scale -- far inside the
2e-2 gate -- and turns the whole cascade into one batched FIR.

The FIR is evaluated on the Tensor engine: for each 128-sample output block
m, out[t, c] = sum_b lhsT_b^T @ taps_b where lhsT_b[p, t] = x[128(m-b) + t +
p - 127] is a 128x128 slice of a precomputed Toeplitz "strip" S[p, u] =
x_pad[u + p] (one overlapping-AP DMA builds it; bf16), and taps_b[p, c] =
h_c[128 b + 127 - p] (constant, bf16).  Bands need 1..6 tap blocks; bands
are grouped by block count so PE work is ~285 psum rows per output block
(~30 us total) instead of 6*128.  PSUM accumulates in f32; each 2 KiB bank
holds 4 output blocks, is copied (f32->bf16) to an SBUF stage by whichever
engine is free, and staged groups of 16 blocks go to DRAM with one DMA
(1 KiB+ descriptors keep the DMA bus at full model bandwidth).

Output DRAM layout is [t_local, m, c] bf16; the host transposes to
[T, C] f32 (host work is not on the device critical path).  Total DMA is
~8.5 MB strip in + ~8 MB out vs 16 MB out alone for the f32 IIR baseline,
and the serial DVE scan chain (8 scans x 32000 cols at 0.96 GHz ~ 270 us)
disappears entirely.

Sharding: batch-parallel SPMD, one waveform per NeuronCore (8 cores, B=8).
"""

import sys

import numpy as np

for _p in ("/opt/trn_rl_repo",):
    if _p not in sys.path:
        sys.path.insert(0, _p)

import ml_dtypes

import concourse.bass as bass  # noqa: F401
import concourse.mybir as mybir
from concourse.bacc import Bacc
from concourse.bass_utils import run_bass_kernel_spmd
from concourse.tile import TileContext

B = 8
T = 32000
C = 128
MB = T // 128            # 250 output blocks of 128 samples
KTAP = 128               # taps per matmul contraction block (<= 128)
NMAX = 6                 # max tap blocks per band
TOL = 7e-3               # tail L2 threshold for per-band tap count
MIN_GROUP = 4            # merge band groups smaller than this
BANK_BLOCKS = 8          # output blocks per PSUM accumulation tile (2 banks)
DMA_BLOCKS = 32          # output blocks staged per output DMA
STAGE_BUFS = 6
TAIL_SIZES = (24, 20, 16, 12, 8, 6, 4, 2)
TAIL_ENGS = ("pool", "sp", "pool", "sp", "pool", "sp", "pool", "sp")
STRIP_PAT = (0, 1, 0, 0, 1, 0, 0, 1, 0, 0, 1, 0, 0)  # 0=SP 1=Act per chunk
BF16 = mybir.dt.bfloat16
F32 = mybir.dt.float32
NPBF16 = ml_dtypes.bfloat16

OFF0 = (NMAX - 1) * KTAP            # strip column of (m=0, b=NMAX-1) window
STRIP_COLS = 128 * (MB - 1) + 128 + OFF0
XPAD_OFF = OFF0 + KTAP - 1          # leading zeros in x_pad
XPAD_LEN = STRIP_COLS + KTAP        # >= STRIP_COLS-1 + (KTAP-1) + 1


def _fir_design(coef_re, coef_im, factor):
    """Exact cascade impulse response h[c, j] and its envelope, j < NMAX*KTAP."""
    cr = np.asarray(coef_re, np.float64)
    ci = np.asarray(coef_im, np.float64)
    f = np.asarray(factor, np.float64)
    lam = np.hypot(cr, ci)
    beta = np.arctan2(ci, cr)
    j = np.arange(NMAX * KTAP, dtype=np.float64)
    cj = (j + 1.0) * (j + 2.0) * (j + 3.0) / 6.0
    env = f[:, None] * cj[None, :] * lam[:, None] ** j[None, :]
    h = env * np.cos(beta[:, None] * j[None, :])
    return h, env


def _plan_groups(env):
    """Per-band tap-block counts -> channel groups [(c0, c1, nblocks)]."""
    tail = np.sqrt((env ** 2)[:, ::-1].cumsum(axis=1))[:, ::-1]
    jreq = (tail > TOL).sum(axis=1)
    nblk = np.clip(np.ceil(jreq / float(KTAP)).astype(int), 1, NMAX)
    # prefix grouping needs nblk non-increasing in c (true for this bank,
    # enforce anyway)
    nblk = np.maximum.accumulate(nblk[::-1])[::-1]
    groups = []
    c0 = 0
    for c in range(1, C + 1):
        if c == C or nblk[c] != nblk[c0]:
            groups.append([c0, c, int(nblk[c0])])
            c0 = c
    # absorb runt groups into a neighbor, keeping the larger block count
    merged = []
    for g in groups:
        if merged and (g[1] - g[0] < MIN_GROUP or merged[-1][1] - merged[-1][0] < MIN_GROUP):
            merged[-1][1] = g[1]
        else:
            merged.append(g)
    return [tuple(g) for g in merged], nblk


def build_bass(groups):
    nc = Bacc()
    xp = nc.declare_dram_parameter("xp", [1, XPAD_LEN], BF16, isOutput=False)
    tp = nc.declare_dram_parameter("taps", [KTAP, NMAX * 128], BF16,
                                   isOutput=False)
    out = nc.declare_dram_parameter("out", [128, MB, C], BF16, isOutput=True)

    with TileContext(nc) as tc:
        with (
            tc.tile_pool(name="consts", bufs=1) as consts,
            tc.tile_pool(name="psum", bufs=32 // BANK_BLOCKS,
                         space="PSUM") as psum_pool,
            tc.tile_pool(name="stage", bufs=STAGE_BUFS) as stage_pool,
        ):
            taps = consts.tile([KTAP, NMAX * 128], BF16, tag="taps",
                               name="taps")
            # taps DMA on the (initially idle) GPSIMD queue, off the
            # SP/Act chains that feed the strip
            nc.gpsimd.dma_start(out=taps[:], in_=tp[:, :])

            # one Toeplitz strip tile, filled by column-range DMAs spread
            # over several engines (the cost model charges a DMA to its
            # issuing engine, so these transfer concurrently); the first
            # range is small so PE's first dependency lands early
            strip = consts.tile([KTAP, STRIP_COLS], BF16, tag="strip",
                                name="strip")
            bounds = [0, 768, 2048]
            while bounds[-1] < STRIP_COLS:
                bounds.append(min(bounds[-1] + 3072, STRIP_COLS))
            for i, (a, bnd) in enumerate(zip(bounds[:-1], bounds[1:])):
                src = bass.AP(xp, a, [[1, KTAP], [1, bnd - a]])
                pat = STRIP_PAT[i] if i < len(STRIP_PAT) else (i % 3 == 1)
                eng = nc.scalar if pat else nc.sync
                eng.dma_start(out=strip[:, a:bnd], in_=src)

            # bulk output groups of DMA_BLOCKS; the tail split finer so the
            # final transfer (and the drain behind it) is short
            # bulk groups of DMA_BLOCKS, then descending sizes so each late
            # transfer is short and flushes right after its data is ready
            tail_sizes = list(TAIL_SIZES)
            sizes = []
            left = MB - sum(tail_sizes)
            while left > 0:
                sizes.append(min(DMA_BLOCKS, left))
                left -= sizes[-1]
            sizes += tail_sizes
            dg = 0
            for gi, mg in enumerate(sizes):
                staged = stage_pool.tile([128, mg, C], BF16, tag="staged",
                                         name="staged")
                for bq in range(0, mg, BANK_BLOCKS):
                    nb = min(BANK_BLOCKS, mg - bq)
                    pt = psum_pool.tile([128, nb, C], F32, tag="bank", name="pt")
                    for ms in range(nb):
                        m = dg + bq + ms
                        for (c0, c1, ng) in groups:
                            for b in range(ng):
                                u0 = 128 * m - KTAP * b + OFF0
                                nc.tensor.matmul(
                                    pt[:, ms, c0:c1],
                                    lhsT=strip[:, u0:u0 + 128],
                                    rhs=taps[:, 128 * b + c0:128 * b + c1],
                                    start=(b == 0),
                                    stop=(b == ng - 1),
                                )
                    nc.any.tensor_copy(staged[:, bq:bq + nb, :], pt[:, :, :])
                # bulk output DMAs ride the GPSIMD queue; late groups
                # alternate GPSIMD/SP so consecutive flushes overlap
                n_tail = len(tail_sizes)
                by_name = {"sp": nc.sync, "pool": nc.gpsimd,
                           "act": nc.scalar}
                if gi >= len(sizes) - n_tail:
                    eng = by_name[TAIL_ENGS[gi - (len(sizes) - n_tail)]]
                else:
                    eng = nc.gpsimd
                eng.dma_start(out=out[:, dg:dg + mg, :], in_=staged[:, :, :])
                dg += mg
    nc.finalize()
    return nc


def make_tables(coef_re, coef_im, factor):
    h, env = _fir_design(coef_re, coef_im, factor)
    groups, nblk = _plan_groups(env)
    nper = np.empty(C, int)
    for c0, c1, ng in groups:
        nper[c0:c1] = ng
    hz = h.copy()
    for c in range(C):
        hz[c, nper[c] * KTAP:] = 0.0
    # tapsT[p, 128*b + c] = hz[c, KTAP*b + (KTAP-1) - p]
    hb = hz.reshape(C, NMAX, KTAP)         # [c, b, j0]
    tapsT = hb[:, :, ::-1].transpose(2, 1, 0).reshape(KTAP, NMAX * C)
    return np.ascontiguousarray(tapsT.astype(NPBF16)), groups


_CACHE = {}


def kernel(inp, coef_re, coef_im, factor):
    inp = np.ascontiguousarray(np.asarray(inp, np.float32))
    assert inp.shape == (B, T)
    tapsT, groups = make_tables(coef_re, coef_im, factor)

    key = tuple(groups)
    if key not in _CACHE:
        _CACHE[key] = build_bass(groups)
    nc = _CACHE[key]

    xpad = np.zeros((B, XPAD_LEN), np.float32)
    xpad[:, XPAD_OFF:XPAD_OFF + T] = inp
    xpad = xpad.astype(NPBF16)

    in_maps = [
        {"xp": xpad[i:i + 1], "taps": tapsT}
        for i in range(B)
    ]
    res = run_bass_kernel_spmd(nc, in_maps, core_ids=list(range(B)))
    out = np.stack([
        np.asarray(res.results[i]["out"]).astype(np.float32)
        .transpose(1, 0, 2).reshape(T, C)
        for i in range(B)
    ])
    return np.ascontiguousarray(out)
